# revision 6
# baseline (speedup 1.0000x reference)
"""Batch 3D-Gaussian rasterizer on 8 Trainium2 NeuronCores.

Strategy: host does the cheap per-gaussian preprocess (projection, conic,
SH color, depth sort) and tile culling; the device does the heavy
pixel x gaussian rasterization. The image is cut into 16x32-pixel tiles;
each (tile, 128-gaussian depth block) becomes one independent job that
computes per-pixel (color partial, invdepth partial, block transmittance).
Jobs are distributed round-robin over the 8 cores; the host then combines
a tile's blocks in depth order with C = C0 + T0*C1 + ..., which is exact.

Per-job device pipeline (gaussian axis on SBUF partitions, pixels on free):
  power  = coef^T @ F            (PE, K=6 quadratic-form features)
  alpha  = exp(power)            (ACT; ln(opacity) folded into coef const)
  alpha_c= min(alpha,.99)*(alpha>=1/255)   (DVE: min, is_ge, mult)
  l      = ln(1 - alpha_c)       (ACT, scale=-1 bias=1 fused)
  cum    = U_incl^T @ l          (PE, triangular-ones inclusive cumsum)
  E      = exp(cum)              (ACT)  # E[g] = T_after(g); T_before(g)=E[g-1]
  out4  += rgbd^T @ T_before - rgbd^T @ E   (PE, signed matmuls; w = Tb-Ta)
  out    = [out4 rows; E[127] = block transmittance row]
"""
import sys
import numpy as np

for _p in ('/opt/trn_rl_repo',):
    if _p not in sys.path:
        sys.path.append(_p)

B, P, H, W = 2, 1024, 128, 128
SCALE_MOD = 1.0
NEAR = 0.2
TS_Y, TS_X = 16, 32          # image tile = 16 rows x 32 cols = 512 px
NPIX = TS_Y * TS_X
GB = 128                     # gaussians per block (SBUF partition dim)
NCORES = 8
NEG_BIG = -1.0e30

_C0 = 0.28209479177387814
_C1 = 0.4886025119029199
_C2 = (1.0925484305920792, -1.0925484305920792, 0.31539156525252005,
       -1.0925484305920792, 0.5462742152960396)
_C3 = (-0.5900435899266435, 2.890611442640554, -0.4570457994644658,
       0.3731763325901154, -0.4570457994644658, 1.445305721320277,
       -0.5900435899266435)


def _eval_sh(sh, dirs):
    x, y, z = dirs[:, 0:1], dirs[:, 1:2], dirs[:, 2:3]
    res = _C0 * sh[:, 0]
    res = res - _C1 * y * sh[:, 1] + _C1 * z * sh[:, 2] - _C1 * x * sh[:, 3]
    xx, yy, zz = x * x, y * y, z * z
    xy, yz, xz = x * y, y * z, x * z
    res = (res + _C2[0] * xy * sh[:, 4] + _C2[1] * yz * sh[:, 5]
           + _C2[2] * (2.0 * zz - xx - yy) * sh[:, 6]
           + _C2[3] * xz * sh[:, 7] + _C2[4] * (xx - yy) * sh[:, 8])
    res = (res + _C3[0] * y * (3.0 * xx - yy) * sh[:, 9]
           + _C3[1] * xy * z * sh[:, 10]
           + _C3[2] * y * (4.0 * zz - xx - yy) * sh[:, 11]
           + _C3[3] * z * (2.0 * zz - 3.0 * xx - 3.0 * yy) * sh[:, 12]
           + _C3[4] * x * (4.0 * zz - xx - yy) * sh[:, 13]
           + _C3[5] * z * (xx - yy) * sh[:, 14]
           + _C3[6] * x * (xx - 3.0 * yy) * sh[:, 15])
    return np.maximum(res + 0.5, 0.0).astype(np.float32)


def _quat_to_rot(q):
    q = q / np.linalg.norm(q, axis=-1, keepdims=True)
    r, x, y, z = q[:, 0], q[:, 1], q[:, 2], q[:, 3]
    return np.stack([
        1 - 2 * (y * y + z * z), 2 * (x * y - r * z), 2 * (x * z + r * y),
        2 * (x * y + r * z), 1 - 2 * (x * x + z * z), 2 * (y * z - r * x),
        2 * (x * z - r * y), 2 * (y * z + r * x), 1 - 2 * (x * x + y * y),
    ], axis=-1).reshape(-1, 3, 3).astype(np.float32)


def _preprocess_one(viewmat, projmat, campos, tanx, tany,
                    means3D, opacities, scales, rotations, sh):
    Pn = means3D.shape[0]
    ones = np.ones((Pn, 1), means3D.dtype)
    p_hom = np.concatenate([means3D, ones], axis=1)
    t = (p_hom @ viewmat)[:, :3]
    p_proj = p_hom @ projmat
    ndc = p_proj[:, :3] / (p_proj[:, 3:4] + 1e-7)
    depth = t[:, 2]

    focal_x = W / (2.0 * tanx)
    focal_y = H / (2.0 * tany)
    tz = np.maximum(depth, 1e-6)
    txc = np.clip(t[:, 0] / tz, -1.3 * tanx, 1.3 * tanx) * tz
    tyc = np.clip(t[:, 1] / tz, -1.3 * tany, 1.3 * tany) * tz
    zero = np.zeros_like(tz)
    J = np.stack([
        np.stack([focal_x / tz, zero, -focal_x * txc / (tz * tz)], -1),
        np.stack([zero, focal_y / tz, -focal_y * tyc / (tz * tz)], -1),
    ], axis=1).astype(np.float32)
    Wr = viewmat[:3, :3].T
    Rq = _quat_to_rot(rotations)
    s2 = ((scales * SCALE_MOD) ** 2).astype(np.float32)
    Sigma = np.einsum('pij,pj,pkj->pik', Rq, s2, Rq).astype(np.float32)
    T = np.einsum('pij,jk->pik', J, Wr).astype(np.float32)
    cov2d = np.einsum('pij,pjk,plk->pil', T, Sigma, T).astype(np.float32)
    a = cov2d[:, 0, 0] + 0.3
    c = cov2d[:, 1, 1] + 0.3
    b = cov2d[:, 0, 1]
    det = a * c - b * b
    valid = (det > 0.0) & (depth > NEAR)
    det_s = np.where(valid, det, 1.0)
    conA, conB, conC = c / det_s, -b / det_s, a / det_s

    mid = 0.5 * (a + c)
    lam = mid + np.sqrt(np.maximum(0.1, mid * mid - det))
    radii = np.where(valid, np.ceil(3.0 * np.sqrt(lam)), 0.0).astype(np.int32)

    px = ((ndc[:, 0] + 1.0) * W - 1.0) * 0.5
    py = ((ndc[:, 1] + 1.0) * H - 1.0) * 0.5

    dirs = means3D - campos[None, :]
    dirs = dirs / np.linalg.norm(dirs, axis=-1, keepdims=True)
    rgb = _eval_sh(sh, dirs)

    order = np.argsort(depth, kind='stable')
    return dict(
        px=px[order], py=py[order],
        cA=conA[order], cB=conB[order], cC=conC[order],
        op=opacities[order, 0], rgb=rgb[order],
        dep=np.maximum(depth[order], 1e-6), valid=valid[order],
        lam=lam[order], radii_unsorted=radii,
    )


def _build_jobs(pps):
    """Returns (job list, packed coef/rgbd arrays). Each job:
    (cam, tile_y, tile_x, block_idx)."""
    nty, ntx = H // TS_Y, W // TS_X
    jobs = []       # (b, ty, tx, blk)
    coefs = []      # [6, GB] f32
    rgbds = []      # [GB, 8] f32
    for b, pp in enumerate(pps):
        op = pp['op'].astype(np.float64)
        ln_t = np.log(np.maximum(op * 255.0, 1e-300))
        r = np.sqrt(2.0 * np.maximum(ln_t, 0.0) * pp['lam'])
        r = np.where((ln_t > 0) & pp['valid'], r, 0.0) + 0.5
        live = pp['valid'] & (ln_t > 0)
        px, py = pp['px'].astype(np.float64), pp['py'].astype(np.float64)
        A = pp['cA'].astype(np.float64)
        Bc = pp['cB'].astype(np.float64)
        C = pp['cC'].astype(np.float64)
        lnop = np.log(np.maximum(op, 1e-300))
        rgbd4 = np.zeros((P, 4), np.float32)
        rgbd4[:, 0:3] = pp['rgb']
        rgbd4[:, 3] = np.float32(1.0) / pp['dep']
        # per-tile membership
        for ty in range(nty):
            y0, y1 = ty * TS_Y, (ty + 1) * TS_Y - 1
            for tx in range(ntx):
                x0, x1 = tx * TS_X, (tx + 1) * TS_X - 1
                sel = np.nonzero(live & (px + r >= x0) & (px - r <= x1)
                                 & (py + r >= y0) & (py - r <= y1))[0]
                if sel.size == 0:
                    continue
                cx = x0 + (TS_X - 1) / 2.0
                cy = y0 + (TS_Y - 1) / 2.0
                pxl, pyl = px[sel] - cx, py[sel] - cy
                As, Bs, Cs = A[sel], Bc[sel], C[sel]
                c6 = np.stack([
                    -0.5 * As,
                    -0.5 * Cs,
                    -Bs,
                    As * pxl + Bs * pyl,
                    Cs * pyl + Bs * pxl,
                    -(0.5 * As * pxl * pxl + 0.5 * Cs * pyl * pyl
                      + Bs * pxl * pyl) + lnop[sel],
                ], axis=0)  # [6, n]
                n = sel.size
                nblk = (n + GB - 1) // GB
                for k in range(nblk):
                    s, e = k * GB, min((k + 1) * GB, n)
                    m = e - s
                    ct = np.zeros((6, GB), np.float32)
                    ct[5, :] = NEG_BIG
                    ct[:, :m] = c6[:, s:e].astype(np.float32)
                    r4 = np.zeros((GB, 4), np.float32)
                    r4[:m] = rgbd4[sel[s:e]]
                    rt = np.zeros((GB, 12), np.float32)
                    rt[0:GB - 1, 0:4] = r4[1:GB]   # shifted: row g = rgbd[g+1]
                    rt[:, 4:8] = -r4
                    rt[0, 8:12] = r4[0]
                    jobs.append((b, ty, tx, k))
                    coefs.append(ct)
                    rgbds.append(rt)
    return jobs, coefs, rgbds


def _features():
    yy, xx = np.meshgrid(np.arange(TS_Y), np.arange(TS_X), indexing='ij')
    xl = (xx - (TS_X - 1) / 2.0).ravel()
    yl = (yy - (TS_Y - 1) / 2.0).ravel()
    return np.stack([xl * xl, yl * yl, xl * yl, xl, yl,
                     np.ones(NPIX)], axis=0).astype(np.float32)  # [6, NPIX]


_NC_CACHE = {}


def _build_nc(J):
    if J in _NC_CACHE:
        return _NC_CACHE[J]
    import concourse.bacc as bacc
    import concourse.mybir as mybir
    import concourse.tile as tile
    from contextlib import ExitStack

    f32 = mybir.dt.float32
    Alu = mybir.AluOpType
    Act = mybir.ActivationFunctionType

    nc = bacc.Bacc("TRN2", target_bir_lowering=False, debug=False,
                   num_devices=NCORES)
    coef_d = nc.dram_tensor("coef", [J, 6, GB], f32, kind="ExternalInput")
    rgbd_d = nc.dram_tensor("rgbd", [J, GB, 12], f32, kind="ExternalInput")
    F_d = nc.dram_tensor("feat", [6, NPIX], f32, kind="ExternalInput")
    U_d = nc.dram_tensor("triu", [GB, GB], f32, kind="ExternalInput")
    out_d = nc.dram_tensor("out", [J, 5, NPIX], f32, kind="ExternalOutput")

    with tile.TileContext(nc) as tc, ExitStack() as ctx:
        const = ctx.enter_context(tc.tile_pool(name="const", bufs=1))
        pool = ctx.enter_context(tc.tile_pool(name="work", bufs=3))
        psum = ctx.enter_context(
            tc.tile_pool(name="psum", bufs=2, space="PSUM"))

        F_t = const.tile([6, NPIX], f32)
        nc.sync.dma_start(F_t[:], F_d[:])
        U_t = const.tile([GB, GB], f32)
        nc.sync.dma_start(U_t[:], U_d[:])
        ones_t = const.tile([1, NPIX], f32)
        nc.gpsimd.memset(ones_t[:], 1.0)

        for j in range(J):
            coef_t = pool.tile([6, GB], f32)
            nc.sync.dma_start(coef_t[:], coef_d[j])
            rgbd_t = pool.tile([GB, 12], f32)
            nc.sync.dma_start(rgbd_t[:], rgbd_d[j])

            power_p = psum.tile([GB, NPIX], f32)
            nc.tensor.matmul(power_p[:], coef_t[:], F_t[:],
                             start=True, stop=True)
            alpha_t = pool.tile([GB, NPIX], f32)
            nc.scalar.activation(alpha_t[:], power_p[:], Act.Exp)
            mask_t = pool.tile([GB, NPIX], f32)
            nc.gpsimd.tensor_scalar(mask_t[:], alpha_t[:], 1.0 / 255.0, None,
                                    Alu.is_ge)
            ac_t = pool.tile([GB, NPIX], f32)
            nc.vector.scalar_tensor_tensor(ac_t[:], alpha_t[:], 0.99,
                                           mask_t[:], Alu.min, Alu.mult)
            l_t = pool.tile([GB, NPIX], f32)
            nc.scalar.activation(l_t[:], ac_t[:], Act.Ln, bias=1.0,
                                 scale=-1.0)
            cum_p = psum.tile([GB, NPIX], f32)
            nc.tensor.matmul(cum_p[:], U_t[:], l_t[:], start=True, stop=True)
            E_t = pool.tile([GB, NPIX], f32)
            nc.scalar.activation(E_t[:], cum_p[:], Act.Exp)

            out_p = psum.tile([4, NPIX], f32)
            nc.tensor.matmul(out_p[:], rgbd_t[0:GB - 1, 0:4], E_t[0:GB - 1, :],
                             start=True, stop=False)
            nc.tensor.matmul(out_p[:], rgbd_t[0:1, 8:12], ones_t[:],
                             start=False, stop=False)
            nc.tensor.matmul(out_p[:], rgbd_t[:, 4:8], E_t[:],
                             start=False, stop=True)
            osb_t = pool.tile([4, NPIX], f32)
            nc.vector.tensor_copy(osb_t[:], out_p[:])
            nc.sync.dma_start(out_d[j, 0:4, :], osb_t[:])
            nc.sync.dma_start(out_d[j, 4:5, :], E_t[GB - 1:GB, :])

    nc.compile()
    _NC_CACHE[J] = nc
    return nc


def kernel(means3D, opacities, scales, rotations, sh, bg,
           viewmatrices, projmatrices, camposes, tanfovxs, tanfovys,
           _run_opts=None):
    from concourse.bass_utils import run_bass_kernel_spmd

    means3D = np.asarray(means3D, np.float32)
    opacities = np.asarray(opacities, np.float32)
    scales = np.asarray(scales, np.float32)
    rotations = np.asarray(rotations, np.float32)
    sh = np.asarray(sh, np.float32)
    bg = np.asarray(bg, np.float32)
    viewmatrices = np.asarray(viewmatrices, np.float32)
    projmatrices = np.asarray(projmatrices, np.float32)
    camposes = np.asarray(camposes, np.float32)
    tanfovxs = np.asarray(tanfovxs, np.float32)
    tanfovys = np.asarray(tanfovys, np.float32)

    pps = [_preprocess_one(viewmatrices[b], projmatrices[b], camposes[b],
                           float(tanfovxs[b]), float(tanfovys[b]),
                           means3D, opacities, scales, rotations, sh)
           for b in range(B)]
    jobs, coefs, rgbds = _build_jobs(pps)

    ncj = len(jobs)
    J = (ncj + NCORES - 1) // NCORES
    # pad with dummy jobs (alpha=0 everywhere)
    dummy_c = np.zeros((6, GB), np.float32)
    dummy_c[5, :] = NEG_BIG
    dummy_r = np.zeros((GB, 12), np.float32)
    while len(jobs) < J * NCORES:
        jobs.append((-1, 0, 0, 0))
        coefs.append(dummy_c)
        rgbds.append(dummy_r)

    F = _features()
    U = np.triu(np.ones((GB, GB), np.float32))  # U[k,m]=1 for k<=m

    nc = _build_nc(J)
    in_maps = []
    for c in range(NCORES):
        sl = slice(c * J, (c + 1) * J)
        in_maps.append({
            'coef': np.stack(coefs[sl.start:sl.stop]),
            'rgbd': np.stack(rgbds[sl.start:sl.stop]),
            'feat': F,
            'triu': U,
        })
    run_opts = dict(_run_opts or {})
    result_sink = run_opts.pop('result_sink', None)
    res = run_bass_kernel_spmd(nc, in_maps, list(range(NCORES)), **run_opts)
    if result_sink is not None:
        result_sink['res'] = res
    outs = [res.results[c]['out'] for c in range(NCORES)]  # [J,5,NPIX] each

    # host combine: per (cam, tile) chain blocks in depth order
    nty, ntx = H // TS_Y, W // TS_X
    acc = {}
    for idx, (b, ty, tx, blk) in enumerate(jobs):
        if b < 0:
            continue
        c, jj = idx // J, idx % J
        acc.setdefault((b, ty, tx), []).append((blk, outs[c][jj]))
    colors = np.zeros((B, 3, H, W), np.float32)
    invd = np.zeros((B, 1, H, W), np.float32)
    for (b, ty, tx), blks in acc.items():
        blks.sort(key=lambda t: t[0])
        Csum = np.zeros((4, NPIX), np.float32)
        Trun = np.ones((NPIX,), np.float32)
        for _, o in blks:
            Csum = Csum + Trun[None, :] * o[0:4]
            Trun = Trun * o[4]
        ybase, xbase = ty * TS_Y, tx * TS_X
        tilec = (Csum[0:3] + Trun[None, :] * bg[:, None]).reshape(3, TS_Y, TS_X)
        colors[b, :, ybase:ybase + TS_Y, xbase:xbase + TS_X] = tilec
        invd[b, 0, ybase:ybase + TS_Y, xbase:xbase + TS_X] = \
            Csum[3].reshape(TS_Y, TS_X)
    # tiles with zero jobs: pure background
    covered = np.zeros((B, nty, ntx), bool)
    for (b, ty, tx) in acc:
        covered[b, ty, tx] = True
    for b in range(B):
        for ty in range(nty):
            for tx in range(ntx):
                if not covered[b, ty, tx]:
                    colors[b, :, ty * TS_Y:(ty + 1) * TS_Y,
                           tx * TS_X:(tx + 1) * TS_X] = bg[:, None, None]

    radii = np.stack([pp['radii_unsorted'] for pp in pps])
    return colors, invd, radii


# revision 13
# speedup vs baseline: 3.6618x; 3.6618x over previous
"""Batch 3D-Gaussian rasterizer on 8 Trainium2 NeuronCores.

Strategy: host does the cheap per-gaussian preprocess (projection, conic,
SH color, depth sort) and tile culling; the device does the heavy
pixel x gaussian rasterization. The image is cut into 16x32-pixel tiles;
each (tile, 128-gaussian depth block) becomes one independent job that
computes per-pixel (color partial, invdepth partial, block transmittance).
Jobs are distributed round-robin over the 8 cores; the host then combines
a tile's blocks in depth order with C = C0 + T0*C1 + ..., which is exact.

Per-job device pipeline (gaussian axis on SBUF partitions, pixels on free):
  power  = coef^T @ F            (PE, K=6 quadratic-form features)
  alpha  = exp(power)            (ACT; ln(opacity) folded into coef const)
  alpha_c= min(alpha,.99)*(alpha>=1/255)   (DVE: min, is_ge, mult)
  l      = ln(1 - alpha_c)       (ACT, scale=-1 bias=1 fused)
  cum    = U_incl^T @ l          (PE, triangular-ones inclusive cumsum)
  E      = exp(cum)              (ACT)  # E[g] = T_after(g); T_before(g)=E[g-1]
  out4  += rgbd^T @ T_before - rgbd^T @ E   (PE, signed matmuls; w = Tb-Ta)
  out    = [out4 rows; E[127] = block transmittance row]
"""
import sys
import numpy as np

for _p in ('/opt/trn_rl_repo',):
    if _p not in sys.path:
        sys.path.append(_p)

B, P, H, W = 2, 1024, 128, 128
SCALE_MOD = 1.0
NEAR = 0.2
TS_Y, TS_X = 16, 16          # image tile = 16x16 = 256 px
NPIX = TS_Y * TS_X
GB = 128                     # SBUF partition dim (16 groups of GRAN rows)
GRAN = 8                     # gaussians per group (tile depth sub-block)
NGRP = GB // GRAN            # groups per job
MOUT = NGRP * 5              # output rows: 4 accums + 1 T row per group
NCORES = 8
NEG_BIG = -1.0e30

_C0 = 0.28209479177387814
_C1 = 0.4886025119029199
_C2 = (1.0925484305920792, -1.0925484305920792, 0.31539156525252005,
       -1.0925484305920792, 0.5462742152960396)
_C3 = (-0.5900435899266435, 2.890611442640554, -0.4570457994644658,
       0.3731763325901154, -0.4570457994644658, 1.445305721320277,
       -0.5900435899266435)


def _eval_sh(sh, dirs):
    x, y, z = dirs[:, 0:1], dirs[:, 1:2], dirs[:, 2:3]
    res = _C0 * sh[:, 0]
    res = res - _C1 * y * sh[:, 1] + _C1 * z * sh[:, 2] - _C1 * x * sh[:, 3]
    xx, yy, zz = x * x, y * y, z * z
    xy, yz, xz = x * y, y * z, x * z
    res = (res + _C2[0] * xy * sh[:, 4] + _C2[1] * yz * sh[:, 5]
           + _C2[2] * (2.0 * zz - xx - yy) * sh[:, 6]
           + _C2[3] * xz * sh[:, 7] + _C2[4] * (xx - yy) * sh[:, 8])
    res = (res + _C3[0] * y * (3.0 * xx - yy) * sh[:, 9]
           + _C3[1] * xy * z * sh[:, 10]
           + _C3[2] * y * (4.0 * zz - xx - yy) * sh[:, 11]
           + _C3[3] * z * (2.0 * zz - 3.0 * xx - 3.0 * yy) * sh[:, 12]
           + _C3[4] * x * (4.0 * zz - xx - yy) * sh[:, 13]
           + _C3[5] * z * (xx - yy) * sh[:, 14]
           + _C3[6] * x * (xx - 3.0 * yy) * sh[:, 15])
    return np.maximum(res + 0.5, 0.0).astype(np.float32)


def _quat_to_rot(q):
    q = q / np.linalg.norm(q, axis=-1, keepdims=True)
    r, x, y, z = q[:, 0], q[:, 1], q[:, 2], q[:, 3]
    return np.stack([
        1 - 2 * (y * y + z * z), 2 * (x * y - r * z), 2 * (x * z + r * y),
        2 * (x * y + r * z), 1 - 2 * (x * x + z * z), 2 * (y * z - r * x),
        2 * (x * z - r * y), 2 * (y * z + r * x), 1 - 2 * (x * x + y * y),
    ], axis=-1).reshape(-1, 3, 3).astype(np.float32)


def _preprocess_one(viewmat, projmat, campos, tanx, tany,
                    means3D, opacities, scales, rotations, sh):
    Pn = means3D.shape[0]
    ones = np.ones((Pn, 1), means3D.dtype)
    p_hom = np.concatenate([means3D, ones], axis=1)
    t = (p_hom @ viewmat)[:, :3]
    p_proj = p_hom @ projmat
    ndc = p_proj[:, :3] / (p_proj[:, 3:4] + 1e-7)
    depth = t[:, 2]

    focal_x = W / (2.0 * tanx)
    focal_y = H / (2.0 * tany)
    tz = np.maximum(depth, 1e-6)
    txc = np.clip(t[:, 0] / tz, -1.3 * tanx, 1.3 * tanx) * tz
    tyc = np.clip(t[:, 1] / tz, -1.3 * tany, 1.3 * tany) * tz
    zero = np.zeros_like(tz)
    J = np.stack([
        np.stack([focal_x / tz, zero, -focal_x * txc / (tz * tz)], -1),
        np.stack([zero, focal_y / tz, -focal_y * tyc / (tz * tz)], -1),
    ], axis=1).astype(np.float32)
    Wr = viewmat[:3, :3].T
    Rq = _quat_to_rot(rotations)
    s2 = ((scales * SCALE_MOD) ** 2).astype(np.float32)
    Sigma = np.einsum('pij,pj,pkj->pik', Rq, s2, Rq).astype(np.float32)
    T = np.einsum('pij,jk->pik', J, Wr).astype(np.float32)
    cov2d = np.einsum('pij,pjk,plk->pil', T, Sigma, T).astype(np.float32)
    a = cov2d[:, 0, 0] + 0.3
    c = cov2d[:, 1, 1] + 0.3
    b = cov2d[:, 0, 1]
    det = a * c - b * b
    valid = (det > 0.0) & (depth > NEAR)
    det_s = np.where(valid, det, 1.0)
    conA, conB, conC = c / det_s, -b / det_s, a / det_s

    mid = 0.5 * (a + c)
    lam = mid + np.sqrt(np.maximum(0.1, mid * mid - det))
    radii = np.where(valid, np.ceil(3.0 * np.sqrt(lam)), 0.0).astype(np.int32)

    px = ((ndc[:, 0] + 1.0) * W - 1.0) * 0.5
    py = ((ndc[:, 1] + 1.0) * H - 1.0) * 0.5

    dirs = means3D - campos[None, :]
    dirs = dirs / np.linalg.norm(dirs, axis=-1, keepdims=True)
    rgb = _eval_sh(sh, dirs)

    order = np.argsort(depth, kind='stable')
    return dict(
        px=px[order], py=py[order],
        cA=conA[order], cB=conB[order], cC=conC[order],
        op=opacities[order, 0], rgb=rgb[order],
        dep=np.maximum(depth[order], 1e-6), valid=valid[order],
        lam=lam[order], radii_unsorted=radii,
    )


def _build_groups(pps):
    """Cut every (cam, tile) gaussian list into depth-ordered chunks of
    GRAN. Returns a list of group dicts: coef [6,GRAN], wd [GRAN,5]
    (diff-rgbd cols 0:4, T-indicator col 4), rfirst [4] (host-added
    constant term), and (b, ty, tx, chunk)."""
    nty, ntx = H // TS_Y, W // TS_X
    groups = []
    for b, pp in enumerate(pps):
        op = pp['op'].astype(np.float64)
        ln_t = np.log(np.maximum(op * 255.0, 1e-300))
        r = np.sqrt(2.0 * np.maximum(ln_t, 0.0) * pp['lam'])
        r = np.where((ln_t > 0) & pp['valid'], r, 0.0) + 0.5
        live = pp['valid'] & (ln_t > 0)
        px, py = pp['px'].astype(np.float64), pp['py'].astype(np.float64)
        A = pp['cA'].astype(np.float64)
        Bc = pp['cB'].astype(np.float64)
        C = pp['cC'].astype(np.float64)
        lnop = np.log(np.maximum(op, 1e-300))
        rgbd4 = np.zeros((P, 4), np.float32)
        rgbd4[:, 0:3] = pp['rgb']
        rgbd4[:, 3] = np.float32(1.0) / pp['dep']
        for ty in range(nty):
            y0, y1 = ty * TS_Y, (ty + 1) * TS_Y - 1
            for tx in range(ntx):
                x0, x1 = tx * TS_X, (tx + 1) * TS_X - 1
                sel = np.nonzero(live & (px + r >= x0) & (px - r <= x1)
                                 & (py + r >= y0) & (py - r <= y1))[0]
                if sel.size == 0:
                    continue
                cx = x0 + (TS_X - 1) / 2.0
                cy = y0 + (TS_Y - 1) / 2.0
                pxl, pyl = px[sel] - cx, py[sel] - cy
                As, Bs, Cs = A[sel], Bc[sel], C[sel]
                c6 = np.stack([
                    -0.5 * As,
                    -0.5 * Cs,
                    -Bs,
                    As * pxl + Bs * pyl,
                    Cs * pyl + Bs * pxl,
                    -(0.5 * As * pxl * pxl + 0.5 * Cs * pyl * pyl
                      + Bs * pxl * pyl) + lnop[sel],
                ], axis=0)  # [6, n]
                n = sel.size
                for k in range((n + GRAN - 1) // GRAN):
                    s, e = k * GRAN, min((k + 1) * GRAN, n)
                    m = e - s
                    ct = np.zeros((6, GRAN), np.float32)
                    ct[5, :] = NEG_BIG
                    ct[:, :m] = c6[:, s:e].astype(np.float32)
                    r4 = np.zeros((GRAN, 4), np.float32)
                    r4[:m] = rgbd4[sel[s:e]]
                    wd = np.zeros((GRAN, 5), np.float32)
                    wd[0:GRAN - 1, 0:4] = r4[1:GRAN] - r4[0:GRAN - 1]
                    wd[GRAN - 1, 0:4] = -r4[GRAN - 1]
                    wd[GRAN - 1, 4] = 1.0   # T row: picks E[last]
                    groups.append(dict(key=(b, ty, tx, k), coef=ct, wd=wd,
                                       rfirst=r4[0].copy()))
    return groups


def _features():
    yy, xx = np.meshgrid(np.arange(TS_Y), np.arange(TS_X), indexing='ij')
    xl = (xx - (TS_X - 1) / 2.0).ravel()
    yl = (yy - (TS_Y - 1) / 2.0).ravel()
    return np.stack([xl * xl, yl * yl, xl * yl, xl, yl,
                     np.ones(NPIX)], axis=0).astype(np.float32)  # [6, NPIX]


_NC_CACHE = {}


def _build_nc(J):
    if J in _NC_CACHE:
        return _NC_CACHE[J]
    import concourse.bacc as bacc
    import concourse.mybir as mybir
    import concourse.tile as tile
    from contextlib import ExitStack

    f32 = mybir.dt.float32
    Alu = mybir.AluOpType
    Act = mybir.ActivationFunctionType

    nc = bacc.Bacc("TRN2", target_bir_lowering=False, debug=False,
                   num_devices=NCORES)
    coef_d = nc.dram_tensor("coef", [J, 6, GB], f32, kind="ExternalInput")
    wout_d = nc.dram_tensor("wout", [J, GB, MOUT], f32, kind="ExternalInput")
    F_d = nc.dram_tensor("feat", [6, NPIX], f32, kind="ExternalInput")
    U_d = nc.dram_tensor("triu", [GB, GB], f32, kind="ExternalInput")
    out_d = nc.dram_tensor("out", [J, MOUT, NPIX], f32, kind="ExternalOutput")

    with tile.TileContext(nc) as tc, ExitStack() as ctx:
        const = ctx.enter_context(tc.tile_pool(name="const", bufs=1))
        pool = ctx.enter_context(tc.tile_pool(name="work", bufs=4))
        # phase-crossing tiles need a slot per job (cheap: J <= ~8)
        lpool = ctx.enter_context(tc.tile_pool(name="lpool", bufs=J))
        epool = ctx.enter_context(tc.tile_pool(name="epool", bufs=J))
        psum = ctx.enter_context(
            tc.tile_pool(name="psum", bufs=2, space="PSUM"))

        F_t = const.tile([6, NPIX], f32)
        nc.sync.dma_start(F_t[:], F_d[:])
        U_t = const.tile([GB, GB], f32)
        nc.sync.dma_start(U_t[:], U_d[:])

        acpool = ctx.enter_context(tc.tile_pool(name="acpool", bufs=J))
        coefs, wouts, alphas, acs, ls, es = ({} for _ in range(6))
        # phase A: feature matmul + exp (one ACT table load)
        for j in range(J):
            coefs[j] = pool.tile([6, GB], f32, tag="coef", name=f"coef{j}")
            nc.sync.dma_start(coefs[j][:], coef_d[j])
            wouts[j] = pool.tile([GB, MOUT], f32, tag="wout", name=f"wout{j}")
            nc.sync.dma_start(wouts[j][:], wout_d[j])
            power_p = psum.tile([GB, NPIX], f32, tag="power")
            nc.tensor.matmul(power_p[:], coefs[j][:], F_t[:],
                             start=True, stop=True)
            alphas[j] = pool.tile([GB, NPIX], f32, tag="alpha", name=f"alpha{j}")
            nc.scalar.activation(alphas[j][:], power_p[:], Act.Exp)
        # phase B: DVE clamp+mask, then Ln (one table load)
        for j in range(J):
            amin_t = pool.tile([GB, NPIX], f32, tag="amin")
            nc.vector.tensor_scalar(amin_t[:], alphas[j][:], 0.99, None,
                                    Alu.min)
            mask_t = pool.tile([GB, NPIX], f32, tag="mask")
            nc.vector.tensor_scalar(mask_t[:], alphas[j][:], 1.0 / 255.0,
                                    None, Alu.is_ge)
            acs[j] = acpool.tile([GB, NPIX], f32, tag="ac", name=f"ac{j}")
            nc.vector.tensor_tensor(acs[j][:], amin_t[:], mask_t[:],
                                    Alu.mult)
        for j in range(J):
            ls[j] = lpool.tile([GB, NPIX], f32, tag="l", name=f"l{j}")
            nc.scalar.activation(ls[j][:], acs[j][:], Act.Ln, bias=1.0,
                                 scale=-1.0)
        # phase C: cumsum matmul + exp (one table load)
        for j in range(J):
            cum_p = psum.tile([GB, NPIX], f32, tag="cum")
            nc.tensor.matmul(cum_p[:], U_t[:], ls[j][:],
                             start=True, stop=True)
            es[j] = epool.tile([GB, NPIX], f32, tag="e", name=f"e{j}")
            nc.scalar.activation(es[j][:], cum_p[:], Act.Exp)
        # phase D: output matmul, copy out, DMA
        for j in range(J):
            out_p = psum.tile([MOUT, NPIX], f32, tag="out")
            nc.tensor.matmul(out_p[:], wouts[j][:], es[j][:],
                             start=True, stop=True)
            osb_t = pool.tile([MOUT, NPIX], f32, tag="osb")
            nc.vector.tensor_copy(osb_t[:], out_p[:])
            nc.sync.dma_start(out_d[j], osb_t[:])

    nc.compile()
    _NC_CACHE[J] = nc
    return nc


def kernel(means3D, opacities, scales, rotations, sh, bg,
           viewmatrices, projmatrices, camposes, tanfovxs, tanfovys,
           _run_opts=None):
    from concourse.bass_utils import run_bass_kernel_spmd

    means3D = np.asarray(means3D, np.float32)
    opacities = np.asarray(opacities, np.float32)
    scales = np.asarray(scales, np.float32)
    rotations = np.asarray(rotations, np.float32)
    sh = np.asarray(sh, np.float32)
    bg = np.asarray(bg, np.float32)
    viewmatrices = np.asarray(viewmatrices, np.float32)
    projmatrices = np.asarray(projmatrices, np.float32)
    camposes = np.asarray(camposes, np.float32)
    tanfovxs = np.asarray(tanfovxs, np.float32)
    tanfovys = np.asarray(tanfovys, np.float32)

    pps = [_preprocess_one(viewmatrices[b], projmatrices[b], camposes[b],
                           float(tanfovxs[b]), float(tanfovys[b]),
                           means3D, opacities, scales, rotations, sh)
           for b in range(B)]
    groups = _build_groups(pps)

    njob = (len(groups) + NGRP - 1) // NGRP
    J = (njob + NCORES - 1) // NCORES
    # pad with dummy groups (alpha=0 everywhere, no output consumer)
    dummy = dict(key=None, coef=None, wd=None, rfirst=None)
    while len(groups) < J * NCORES * NGRP:
        groups.append(dummy)

    F = _features()
    U8 = np.triu(np.ones((GRAN, GRAN), np.float32))
    U = np.zeros((GB, GB), np.float32)
    for g in range(NGRP):
        U[g * GRAN:(g + 1) * GRAN, g * GRAN:(g + 1) * GRAN] = U8

    # pack per (core, job): coef [6,GB], wout [GB,MOUT]
    coef_all = np.zeros((NCORES, J, 6, GB), np.float32)
    coef_all[:, :, 5, :] = NEG_BIG
    wout_all = np.zeros((NCORES, J, GB, MOUT), np.float32)
    gmeta = {}  # (core, j, g) -> (key, rfirst)
    for i, gr in enumerate(groups):
        cj, g = divmod(i, NGRP)
        c, j = divmod(cj, J)
        if gr['key'] is None:
            continue
        coef_all[c, j, :, g * GRAN:(g + 1) * GRAN] = gr['coef']
        wout_all[c, j, g * GRAN:(g + 1) * GRAN, g * 5:(g + 1) * 5] = gr['wd']
        gmeta[(c, j, g)] = (gr['key'], gr['rfirst'])

    nc = _build_nc(J)
    in_maps = [{'coef': coef_all[c], 'wout': wout_all[c],
                'feat': F, 'triu': U} for c in range(NCORES)]
    run_opts = dict(_run_opts or {})
    result_sink = run_opts.pop('result_sink', None)
    res = run_bass_kernel_spmd(nc, in_maps, list(range(NCORES)), **run_opts)
    if result_sink is not None:
        result_sink['res'] = res
    outs = [res.results[c]['out'] for c in range(NCORES)]  # [J,MOUT,NPIX]

    # host combine: per (cam, tile) chain chunks in depth order
    nty, ntx = H // TS_Y, W // TS_X
    acc = {}
    for (c, j, g), (key, rfirst) in gmeta.items():
        b, ty, tx, chunk = key
        o = outs[c][j][g * 5:(g + 1) * 5]          # [5, NPIX]
        acc.setdefault((b, ty, tx), []).append((chunk, o, rfirst))
    colors = np.zeros((B, 3, H, W), np.float32)
    invd = np.zeros((B, 1, H, W), np.float32)
    for b in range(B):
        colors[b] = bg[:, None, None]
    for (b, ty, tx), chunks in acc.items():
        chunks.sort(key=lambda t: t[0])
        Csum = np.zeros((4, NPIX), np.float32)
        Trun = np.ones((NPIX,), np.float32)
        for _, o, rfirst in chunks:
            Csum = Csum + Trun[None, :] * (o[0:4] + rfirst[:, None])
            Trun = Trun * o[4]
        ybase, xbase = ty * TS_Y, tx * TS_X
        tilec = (Csum[0:3] + Trun[None, :] * bg[:, None]).reshape(
            3, TS_Y, TS_X)
        colors[b, :, ybase:ybase + TS_Y, xbase:xbase + TS_X] = tilec
        invd[b, 0, ybase:ybase + TS_Y, xbase:xbase + TS_X] = \
            Csum[3].reshape(TS_Y, TS_X)

    radii = np.stack([pp['radii_unsorted'] for pp in pps])
    return colors, invd, radii


# revision 14
# speedup vs baseline: 4.0327x; 1.1013x over previous
"""Batch 3D-Gaussian rasterizer on 8 Trainium2 NeuronCores.

Strategy: host does the cheap per-gaussian preprocess (projection, conic,
SH color, depth sort) and tile culling; the device does the heavy
pixel x gaussian rasterization. The image is cut into 16x32-pixel tiles;
each (tile, 128-gaussian depth block) becomes one independent job that
computes per-pixel (color partial, invdepth partial, block transmittance).
Jobs are distributed round-robin over the 8 cores; the host then combines
a tile's blocks in depth order with C = C0 + T0*C1 + ..., which is exact.

Per-job device pipeline (gaussian axis on SBUF partitions, pixels on free):
  power  = coef^T @ F            (PE, K=6 quadratic-form features)
  alpha  = exp(power)            (ACT; ln(opacity) folded into coef const)
  alpha_c= min(alpha,.99)*(alpha>=1/255)   (DVE: min, is_ge, mult)
  l      = ln(1 - alpha_c)       (ACT, scale=-1 bias=1 fused)
  cum    = U_incl^T @ l          (PE, triangular-ones inclusive cumsum)
  E      = exp(cum)              (ACT)  # E[g] = T_after(g); T_before(g)=E[g-1]
  out4  += rgbd^T @ T_before - rgbd^T @ E   (PE, signed matmuls; w = Tb-Ta)
  out    = [out4 rows; E[127] = block transmittance row]
"""
import sys
import numpy as np

for _p in ('/opt/trn_rl_repo',):
    if _p not in sys.path:
        sys.path.append(_p)

B, P, H, W = 2, 1024, 128, 128
SCALE_MOD = 1.0
NEAR = 0.2
TS_Y, TS_X = 16, 16          # image tile = 16x16 = 256 px
NPIX = TS_Y * TS_X
GB = 128                     # SBUF partition dim (16 groups of GRAN rows)
GRAN = 8                     # gaussians per group (tile depth sub-block)
NGRP = GB // GRAN            # groups per job
MOUT = NGRP * 5              # output rows: 4 accums + 1 T row per group
NCORES = 8
NEG_BIG = -1.0e30

_C0 = 0.28209479177387814
_C1 = 0.4886025119029199
_C2 = (1.0925484305920792, -1.0925484305920792, 0.31539156525252005,
       -1.0925484305920792, 0.5462742152960396)
_C3 = (-0.5900435899266435, 2.890611442640554, -0.4570457994644658,
       0.3731763325901154, -0.4570457994644658, 1.445305721320277,
       -0.5900435899266435)


def _eval_sh(sh, dirs):
    x, y, z = dirs[:, 0:1], dirs[:, 1:2], dirs[:, 2:3]
    res = _C0 * sh[:, 0]
    res = res - _C1 * y * sh[:, 1] + _C1 * z * sh[:, 2] - _C1 * x * sh[:, 3]
    xx, yy, zz = x * x, y * y, z * z
    xy, yz, xz = x * y, y * z, x * z
    res = (res + _C2[0] * xy * sh[:, 4] + _C2[1] * yz * sh[:, 5]
           + _C2[2] * (2.0 * zz - xx - yy) * sh[:, 6]
           + _C2[3] * xz * sh[:, 7] + _C2[4] * (xx - yy) * sh[:, 8])
    res = (res + _C3[0] * y * (3.0 * xx - yy) * sh[:, 9]
           + _C3[1] * xy * z * sh[:, 10]
           + _C3[2] * y * (4.0 * zz - xx - yy) * sh[:, 11]
           + _C3[3] * z * (2.0 * zz - 3.0 * xx - 3.0 * yy) * sh[:, 12]
           + _C3[4] * x * (4.0 * zz - xx - yy) * sh[:, 13]
           + _C3[5] * z * (xx - yy) * sh[:, 14]
           + _C3[6] * x * (xx - 3.0 * yy) * sh[:, 15])
    return np.maximum(res + 0.5, 0.0).astype(np.float32)


def _quat_to_rot(q):
    q = q / np.linalg.norm(q, axis=-1, keepdims=True)
    r, x, y, z = q[:, 0], q[:, 1], q[:, 2], q[:, 3]
    return np.stack([
        1 - 2 * (y * y + z * z), 2 * (x * y - r * z), 2 * (x * z + r * y),
        2 * (x * y + r * z), 1 - 2 * (x * x + z * z), 2 * (y * z - r * x),
        2 * (x * z - r * y), 2 * (y * z + r * x), 1 - 2 * (x * x + y * y),
    ], axis=-1).reshape(-1, 3, 3).astype(np.float32)


def _preprocess_one(viewmat, projmat, campos, tanx, tany,
                    means3D, opacities, scales, rotations, sh):
    Pn = means3D.shape[0]
    ones = np.ones((Pn, 1), means3D.dtype)
    p_hom = np.concatenate([means3D, ones], axis=1)
    t = (p_hom @ viewmat)[:, :3]
    p_proj = p_hom @ projmat
    ndc = p_proj[:, :3] / (p_proj[:, 3:4] + 1e-7)
    depth = t[:, 2]

    focal_x = W / (2.0 * tanx)
    focal_y = H / (2.0 * tany)
    tz = np.maximum(depth, 1e-6)
    txc = np.clip(t[:, 0] / tz, -1.3 * tanx, 1.3 * tanx) * tz
    tyc = np.clip(t[:, 1] / tz, -1.3 * tany, 1.3 * tany) * tz
    zero = np.zeros_like(tz)
    J = np.stack([
        np.stack([focal_x / tz, zero, -focal_x * txc / (tz * tz)], -1),
        np.stack([zero, focal_y / tz, -focal_y * tyc / (tz * tz)], -1),
    ], axis=1).astype(np.float32)
    Wr = viewmat[:3, :3].T
    Rq = _quat_to_rot(rotations)
    s2 = ((scales * SCALE_MOD) ** 2).astype(np.float32)
    Sigma = np.einsum('pij,pj,pkj->pik', Rq, s2, Rq).astype(np.float32)
    T = np.einsum('pij,jk->pik', J, Wr).astype(np.float32)
    cov2d = np.einsum('pij,pjk,plk->pil', T, Sigma, T).astype(np.float32)
    a = cov2d[:, 0, 0] + 0.3
    c = cov2d[:, 1, 1] + 0.3
    b = cov2d[:, 0, 1]
    det = a * c - b * b
    valid = (det > 0.0) & (depth > NEAR)
    det_s = np.where(valid, det, 1.0)
    conA, conB, conC = c / det_s, -b / det_s, a / det_s

    mid = 0.5 * (a + c)
    lam = mid + np.sqrt(np.maximum(0.1, mid * mid - det))
    radii = np.where(valid, np.ceil(3.0 * np.sqrt(lam)), 0.0).astype(np.int32)

    px = ((ndc[:, 0] + 1.0) * W - 1.0) * 0.5
    py = ((ndc[:, 1] + 1.0) * H - 1.0) * 0.5

    dirs = means3D - campos[None, :]
    dirs = dirs / np.linalg.norm(dirs, axis=-1, keepdims=True)
    rgb = _eval_sh(sh, dirs)

    order = np.argsort(depth, kind='stable')
    return dict(
        px=px[order], py=py[order],
        cA=conA[order], cB=conB[order], cC=conC[order],
        op=opacities[order, 0], rgb=rgb[order],
        dep=np.maximum(depth[order], 1e-6), valid=valid[order],
        lam=lam[order], radii_unsorted=radii,
    )


def _build_groups(pps):
    """Cut every (cam, tile) gaussian list into depth-ordered chunks of
    GRAN. Returns a list of group dicts: coef [6,GRAN], wd [GRAN,5]
    (diff-rgbd cols 0:4, T-indicator col 4), rfirst [4] (host-added
    constant term), and (b, ty, tx, chunk)."""
    nty, ntx = H // TS_Y, W // TS_X
    groups = []
    for b, pp in enumerate(pps):
        op = pp['op'].astype(np.float64)
        ln_t = np.log(np.maximum(op * 255.0, 1e-300))
        r = np.sqrt(2.0 * np.maximum(ln_t, 0.0) * pp['lam'])
        r = np.where((ln_t > 0) & pp['valid'], r, 0.0) + 0.5
        live = pp['valid'] & (ln_t > 0)
        px, py = pp['px'].astype(np.float64), pp['py'].astype(np.float64)
        A = pp['cA'].astype(np.float64)
        Bc = pp['cB'].astype(np.float64)
        C = pp['cC'].astype(np.float64)
        lnop = np.log(np.maximum(op, 1e-300))
        rgbd4 = np.zeros((P, 4), np.float32)
        rgbd4[:, 0:3] = pp['rgb']
        rgbd4[:, 3] = np.float32(1.0) / pp['dep']
        for ty in range(nty):
            y0, y1 = ty * TS_Y, (ty + 1) * TS_Y - 1
            for tx in range(ntx):
                x0, x1 = tx * TS_X, (tx + 1) * TS_X - 1
                sel = np.nonzero(live & (px + r >= x0) & (px - r <= x1)
                                 & (py + r >= y0) & (py - r <= y1))[0]
                if sel.size == 0:
                    continue
                cx = x0 + (TS_X - 1) / 2.0
                cy = y0 + (TS_Y - 1) / 2.0
                pxl, pyl = px[sel] - cx, py[sel] - cy
                As, Bs, Cs = A[sel], Bc[sel], C[sel]
                c6 = np.stack([
                    -0.5 * As,
                    -0.5 * Cs,
                    -Bs,
                    As * pxl + Bs * pyl,
                    Cs * pyl + Bs * pxl,
                    -(0.5 * As * pxl * pxl + 0.5 * Cs * pyl * pyl
                      + Bs * pxl * pyl) + lnop[sel],
                ], axis=0)  # [6, n]
                n = sel.size
                for k in range((n + GRAN - 1) // GRAN):
                    s, e = k * GRAN, min((k + 1) * GRAN, n)
                    m = e - s
                    ct = np.zeros((6, GRAN), np.float32)
                    ct[5, :] = NEG_BIG
                    ct[:, :m] = c6[:, s:e].astype(np.float32)
                    r4 = np.zeros((GRAN, 4), np.float32)
                    r4[:m] = rgbd4[sel[s:e]]
                    wd = np.zeros((GRAN, 5), np.float32)
                    wd[0:GRAN - 1, 0:4] = r4[1:GRAN] - r4[0:GRAN - 1]
                    wd[GRAN - 1, 0:4] = -r4[GRAN - 1]
                    wd[GRAN - 1, 4] = 1.0   # T row: picks E[last]
                    groups.append(dict(key=(b, ty, tx, k), coef=ct, wd=wd,
                                       rfirst=r4[0].copy()))
    return groups


def _features():
    yy, xx = np.meshgrid(np.arange(TS_Y), np.arange(TS_X), indexing='ij')
    xl = (xx - (TS_X - 1) / 2.0).ravel()
    yl = (yy - (TS_Y - 1) / 2.0).ravel()
    return np.stack([xl * xl, yl * yl, xl * yl, xl, yl,
                     np.ones(NPIX)], axis=0).astype(np.float32)  # [6, NPIX]


_NC_CACHE = {}


def _build_nc(J):
    if J in _NC_CACHE:
        return _NC_CACHE[J]
    import concourse.bacc as bacc
    import concourse.mybir as mybir
    import concourse.tile as tile
    from concourse.tile import add_dep_helper
    from contextlib import ExitStack

    f32 = mybir.dt.float32
    Alu = mybir.AluOpType
    Act = mybir.ActivationFunctionType

    nc = bacc.Bacc("TRN2", target_bir_lowering=False, debug=False,
                   num_devices=NCORES)
    # batched layouts: one DMA each for coef / wout / out
    coef_d = nc.dram_tensor("coef", [J, 6, GB], f32, kind="ExternalInput")
    wout_d = nc.dram_tensor("wout", [J, GB, MOUT], f32, kind="ExternalInput")
    F_d = nc.dram_tensor("feat", [6, NPIX], f32, kind="ExternalInput")
    U_d = nc.dram_tensor("triu", [GB, GB], f32, kind="ExternalInput")
    out_d = nc.dram_tensor("out", [MOUT, J, NPIX], f32, kind="ExternalOutput")

    with tile.TileContext(nc) as tc, ExitStack() as ctx:
        const = ctx.enter_context(tc.tile_pool(name="const", bufs=1))
        pool = ctx.enter_context(tc.tile_pool(name="work", bufs=4))
        lpool = ctx.enter_context(tc.tile_pool(name="lpool", bufs=J))
        epool = ctx.enter_context(tc.tile_pool(name="epool", bufs=J))
        acpool = ctx.enter_context(tc.tile_pool(name="acpool", bufs=J))
        psum = ctx.enter_context(
            tc.tile_pool(name="psum", bufs=2, space="PSUM"))

        F_t = const.tile([6, NPIX], f32)
        nc.sync.dma_start(F_t[:], F_d[:])
        U_t = const.tile([GB, GB], f32)
        nc.sync.dma_start(U_t[:], U_d[:])
        # all jobs' coefs in one tile/DMA: [6, J*GB]
        coef_t = const.tile([6, J * GB], f32)
        nc.sync.dma_start(
            coef_t[:].rearrange("p (j g) -> p j g", j=J),
            coef_d[:].rearrange("j p g -> p j g"))
        # all jobs' output weights in one tile/DMA: [GB, J*MOUT]
        wout_t = const.tile([GB, J * MOUT], f32)
        nc.sync.dma_start(
            wout_t[:].rearrange("p (j m) -> p j m", j=J),
            wout_d[:].rearrange("j p m -> p j m"))
        # batched output staging
        oall_t = const.tile([MOUT, J * NPIX], f32)

        alphas, acs, ls, es = ({} for _ in range(4))
        expA, lns, expC = [], [], []
        # phase A: feature matmul + exp (one ACT table load)
        for j in range(J):
            power_p = psum.tile([GB, NPIX], f32, tag="power")
            nc.tensor.matmul(power_p[:], coef_t[:, j * GB:(j + 1) * GB],
                             F_t[:], start=True, stop=True)
            alphas[j] = pool.tile([GB, NPIX], f32, tag="alpha",
                                  name=f"alpha{j}")
            expA.append(nc.scalar.activation(alphas[j][:], power_p[:],
                                             Act.Exp))
        # phase B: DVE clamp+mask, then Ln (one table load)
        for j in range(J):
            amin_t = pool.tile([GB, NPIX], f32, tag="amin")
            nc.vector.tensor_scalar(amin_t[:], alphas[j][:], 0.99, None,
                                    Alu.min)
            mask_t = pool.tile([GB, NPIX], f32, tag="mask")
            nc.vector.tensor_scalar(mask_t[:], alphas[j][:], 1.0 / 255.0,
                                    None, Alu.is_ge)
            acs[j] = acpool.tile([GB, NPIX], f32, tag="ac", name=f"ac{j}")
            nc.vector.tensor_tensor(acs[j][:], amin_t[:], mask_t[:],
                                    Alu.mult)
        for j in range(J):
            ls[j] = lpool.tile([GB, NPIX], f32, tag="l", name=f"l{j}")
            i = nc.scalar.activation(ls[j][:], acs[j][:], Act.Ln, bias=1.0,
                                     scale=-1.0)
            lns.append(i)
            add_dep_helper(i.ins, expA[-1].ins, sync=False,
                           reason="group ACT tables: Ln after phase-A Exps")
        # phase C: cumsum matmul + exp (one table load)
        for j in range(J):
            cum_p = psum.tile([GB, NPIX], f32, tag="cum")
            nc.tensor.matmul(cum_p[:], U_t[:], ls[j][:],
                             start=True, stop=True)
            es[j] = epool.tile([GB, NPIX], f32, tag="e", name=f"e{j}")
            i = nc.scalar.activation(es[j][:], cum_p[:], Act.Exp)
            expC.append(i)
            add_dep_helper(i.ins, lns[-1].ins, sync=False,
                           reason="group ACT tables: Exp after all Ln")
        # phase D: output matmul, copy into batched staging, one DMA
        for j in range(J):
            out_p = psum.tile([MOUT, NPIX], f32, tag="out")
            nc.tensor.matmul(out_p[:], wout_t[:, j * MOUT:(j + 1) * MOUT],
                             es[j][:], start=True, stop=True)
            nc.vector.tensor_copy(oall_t[:, j * NPIX:(j + 1) * NPIX],
                                  out_p[:])
        nc.sync.dma_start(
            out_d[:].rearrange("p j n -> p (j n)"), oall_t[:])

    nc.compile()
    _NC_CACHE[J] = nc
    return nc


def kernel(means3D, opacities, scales, rotations, sh, bg,
           viewmatrices, projmatrices, camposes, tanfovxs, tanfovys,
           _run_opts=None):
    from concourse.bass_utils import run_bass_kernel_spmd

    means3D = np.asarray(means3D, np.float32)
    opacities = np.asarray(opacities, np.float32)
    scales = np.asarray(scales, np.float32)
    rotations = np.asarray(rotations, np.float32)
    sh = np.asarray(sh, np.float32)
    bg = np.asarray(bg, np.float32)
    viewmatrices = np.asarray(viewmatrices, np.float32)
    projmatrices = np.asarray(projmatrices, np.float32)
    camposes = np.asarray(camposes, np.float32)
    tanfovxs = np.asarray(tanfovxs, np.float32)
    tanfovys = np.asarray(tanfovys, np.float32)

    pps = [_preprocess_one(viewmatrices[b], projmatrices[b], camposes[b],
                           float(tanfovxs[b]), float(tanfovys[b]),
                           means3D, opacities, scales, rotations, sh)
           for b in range(B)]
    groups = _build_groups(pps)

    njob = (len(groups) + NGRP - 1) // NGRP
    J = (njob + NCORES - 1) // NCORES
    # pad with dummy groups (alpha=0 everywhere, no output consumer)
    dummy = dict(key=None, coef=None, wd=None, rfirst=None)
    while len(groups) < J * NCORES * NGRP:
        groups.append(dummy)

    F = _features()
    U8 = np.triu(np.ones((GRAN, GRAN), np.float32))
    U = np.zeros((GB, GB), np.float32)
    for g in range(NGRP):
        U[g * GRAN:(g + 1) * GRAN, g * GRAN:(g + 1) * GRAN] = U8

    # pack per (core, job): coef [6,GB], wout [GB,MOUT]
    coef_all = np.zeros((NCORES, J, 6, GB), np.float32)
    coef_all[:, :, 5, :] = NEG_BIG
    wout_all = np.zeros((NCORES, J, GB, MOUT), np.float32)
    gmeta = {}  # (core, j, g) -> (key, rfirst)
    for i, gr in enumerate(groups):
        cj, g = divmod(i, NGRP)
        c, j = divmod(cj, J)
        if gr['key'] is None:
            continue
        coef_all[c, j, :, g * GRAN:(g + 1) * GRAN] = gr['coef']
        wout_all[c, j, g * GRAN:(g + 1) * GRAN, g * 5:(g + 1) * 5] = gr['wd']
        gmeta[(c, j, g)] = (gr['key'], gr['rfirst'])

    nc = _build_nc(J)
    in_maps = [{'coef': coef_all[c], 'wout': wout_all[c],
                'feat': F, 'triu': U} for c in range(NCORES)]
    run_opts = dict(_run_opts or {})
    result_sink = run_opts.pop('result_sink', None)
    res = run_bass_kernel_spmd(nc, in_maps, list(range(NCORES)), **run_opts)
    if result_sink is not None:
        result_sink['res'] = res
    outs = [res.results[c]['out'] for c in range(NCORES)]  # [MOUT,J,NPIX]

    # host combine: per (cam, tile) chain chunks in depth order
    nty, ntx = H // TS_Y, W // TS_X
    acc = {}
    for (c, j, g), (key, rfirst) in gmeta.items():
        b, ty, tx, chunk = key
        o = outs[c][g * 5:(g + 1) * 5, j]          # [5, NPIX]
        acc.setdefault((b, ty, tx), []).append((chunk, o, rfirst))
    colors = np.zeros((B, 3, H, W), np.float32)
    invd = np.zeros((B, 1, H, W), np.float32)
    for b in range(B):
        colors[b] = bg[:, None, None]
    for (b, ty, tx), chunks in acc.items():
        chunks.sort(key=lambda t: t[0])
        Csum = np.zeros((4, NPIX), np.float32)
        Trun = np.ones((NPIX,), np.float32)
        for _, o, rfirst in chunks:
            Csum = Csum + Trun[None, :] * (o[0:4] + rfirst[:, None])
            Trun = Trun * o[4]
        ybase, xbase = ty * TS_Y, tx * TS_X
        tilec = (Csum[0:3] + Trun[None, :] * bg[:, None]).reshape(
            3, TS_Y, TS_X)
        colors[b, :, ybase:ybase + TS_Y, xbase:xbase + TS_X] = tilec
        invd[b, 0, ybase:ybase + TS_Y, xbase:xbase + TS_X] = \
            Csum[3].reshape(TS_Y, TS_X)

    radii = np.stack([pp['radii_unsorted'] for pp in pps])
    return colors, invd, radii


# revision 16
# speedup vs baseline: 4.5808x; 1.1359x over previous
"""Batch 3D-Gaussian rasterizer on 8 Trainium2 NeuronCores.

Strategy: host does the cheap per-gaussian preprocess (projection, conic,
SH color, depth sort) and tile culling; the device does the heavy
pixel x gaussian rasterization. The image is cut into 16x32-pixel tiles;
each (tile, 128-gaussian depth block) becomes one independent job that
computes per-pixel (color partial, invdepth partial, block transmittance).
Jobs are distributed round-robin over the 8 cores; the host then combines
a tile's blocks in depth order with C = C0 + T0*C1 + ..., which is exact.

Per-job device pipeline (gaussian axis on SBUF partitions, pixels on free):
  power  = coef^T @ F            (PE, K=6 quadratic-form features)
  alpha  = exp(power)            (ACT; ln(opacity) folded into coef const)
  alpha_c= min(alpha,.99)*(alpha>=1/255)   (DVE: min, is_ge, mult)
  l      = ln(1 - alpha_c)       (ACT, scale=-1 bias=1 fused)
  cum    = U_incl^T @ l          (PE, triangular-ones inclusive cumsum)
  E      = exp(cum)              (ACT)  # E[g] = T_after(g); T_before(g)=E[g-1]
  out4  += rgbd^T @ T_before - rgbd^T @ E   (PE, signed matmuls; w = Tb-Ta)
  out    = [out4 rows; E[127] = block transmittance row]
"""
import sys
import numpy as np

for _p in ('/opt/trn_rl_repo',):
    if _p not in sys.path:
        sys.path.append(_p)

B, P, H, W = 2, 1024, 128, 128
SCALE_MOD = 1.0
NEAR = 0.2
TS_Y, TS_X = 16, 16          # image tile = 16x16 = 256 px
NPIX = TS_Y * TS_X
GB = 128                     # SBUF partition dim (16 groups of GRAN rows)
GRAN = 8                     # gaussians per group (tile depth sub-block)
NGRP = GB // GRAN            # groups per job
MOUT = NGRP * 5              # output rows: 4 accums + 1 T row per group
NCORES = 8
NEG_BIG = -1.0e30

_C0 = 0.28209479177387814
_C1 = 0.4886025119029199
_C2 = (1.0925484305920792, -1.0925484305920792, 0.31539156525252005,
       -1.0925484305920792, 0.5462742152960396)
_C3 = (-0.5900435899266435, 2.890611442640554, -0.4570457994644658,
       0.3731763325901154, -0.4570457994644658, 1.445305721320277,
       -0.5900435899266435)


def _eval_sh(sh, dirs):
    x, y, z = dirs[:, 0:1], dirs[:, 1:2], dirs[:, 2:3]
    res = _C0 * sh[:, 0]
    res = res - _C1 * y * sh[:, 1] + _C1 * z * sh[:, 2] - _C1 * x * sh[:, 3]
    xx, yy, zz = x * x, y * y, z * z
    xy, yz, xz = x * y, y * z, x * z
    res = (res + _C2[0] * xy * sh[:, 4] + _C2[1] * yz * sh[:, 5]
           + _C2[2] * (2.0 * zz - xx - yy) * sh[:, 6]
           + _C2[3] * xz * sh[:, 7] + _C2[4] * (xx - yy) * sh[:, 8])
    res = (res + _C3[0] * y * (3.0 * xx - yy) * sh[:, 9]
           + _C3[1] * xy * z * sh[:, 10]
           + _C3[2] * y * (4.0 * zz - xx - yy) * sh[:, 11]
           + _C3[3] * z * (2.0 * zz - 3.0 * xx - 3.0 * yy) * sh[:, 12]
           + _C3[4] * x * (4.0 * zz - xx - yy) * sh[:, 13]
           + _C3[5] * z * (xx - yy) * sh[:, 14]
           + _C3[6] * x * (xx - 3.0 * yy) * sh[:, 15])
    return np.maximum(res + 0.5, 0.0).astype(np.float32)


def _quat_to_rot(q):
    q = q / np.linalg.norm(q, axis=-1, keepdims=True)
    r, x, y, z = q[:, 0], q[:, 1], q[:, 2], q[:, 3]
    return np.stack([
        1 - 2 * (y * y + z * z), 2 * (x * y - r * z), 2 * (x * z + r * y),
        2 * (x * y + r * z), 1 - 2 * (x * x + z * z), 2 * (y * z - r * x),
        2 * (x * z - r * y), 2 * (y * z + r * x), 1 - 2 * (x * x + y * y),
    ], axis=-1).reshape(-1, 3, 3).astype(np.float32)


def _preprocess_one(viewmat, projmat, campos, tanx, tany,
                    means3D, opacities, scales, rotations, sh):
    Pn = means3D.shape[0]
    ones = np.ones((Pn, 1), means3D.dtype)
    p_hom = np.concatenate([means3D, ones], axis=1)
    t = (p_hom @ viewmat)[:, :3]
    p_proj = p_hom @ projmat
    ndc = p_proj[:, :3] / (p_proj[:, 3:4] + 1e-7)
    depth = t[:, 2]

    focal_x = W / (2.0 * tanx)
    focal_y = H / (2.0 * tany)
    tz = np.maximum(depth, 1e-6)
    txc = np.clip(t[:, 0] / tz, -1.3 * tanx, 1.3 * tanx) * tz
    tyc = np.clip(t[:, 1] / tz, -1.3 * tany, 1.3 * tany) * tz
    zero = np.zeros_like(tz)
    J = np.stack([
        np.stack([focal_x / tz, zero, -focal_x * txc / (tz * tz)], -1),
        np.stack([zero, focal_y / tz, -focal_y * tyc / (tz * tz)], -1),
    ], axis=1).astype(np.float32)
    Wr = viewmat[:3, :3].T
    Rq = _quat_to_rot(rotations)
    s2 = ((scales * SCALE_MOD) ** 2).astype(np.float32)
    Sigma = np.einsum('pij,pj,pkj->pik', Rq, s2, Rq).astype(np.float32)
    T = np.einsum('pij,jk->pik', J, Wr).astype(np.float32)
    cov2d = np.einsum('pij,pjk,plk->pil', T, Sigma, T).astype(np.float32)
    a = cov2d[:, 0, 0] + 0.3
    c = cov2d[:, 1, 1] + 0.3
    b = cov2d[:, 0, 1]
    det = a * c - b * b
    valid = (det > 0.0) & (depth > NEAR)
    det_s = np.where(valid, det, 1.0)
    conA, conB, conC = c / det_s, -b / det_s, a / det_s

    mid = 0.5 * (a + c)
    lam = mid + np.sqrt(np.maximum(0.1, mid * mid - det))
    radii = np.where(valid, np.ceil(3.0 * np.sqrt(lam)), 0.0).astype(np.int32)

    px = ((ndc[:, 0] + 1.0) * W - 1.0) * 0.5
    py = ((ndc[:, 1] + 1.0) * H - 1.0) * 0.5

    dirs = means3D - campos[None, :]
    dirs = dirs / np.linalg.norm(dirs, axis=-1, keepdims=True)
    rgb = _eval_sh(sh, dirs)

    order = np.argsort(depth, kind='stable')
    return dict(
        px=px[order], py=py[order],
        cA=conA[order], cB=conB[order], cC=conC[order],
        op=opacities[order, 0], rgb=rgb[order],
        dep=np.maximum(depth[order], 1e-6), valid=valid[order],
        lam=lam[order], radii_unsorted=radii,
    )


def _build_groups(pps):
    """Cut every (cam, tile) gaussian list into depth-ordered chunks of
    GRAN. Returns a list of group dicts: coef [6,GRAN], wd [GRAN,5]
    (diff-rgbd cols 0:4, T-indicator col 4), rfirst [4] (host-added
    constant term), and (b, ty, tx, chunk)."""
    nty, ntx = H // TS_Y, W // TS_X
    groups = []
    for b, pp in enumerate(pps):
        op = pp['op'].astype(np.float64)
        ln_t = np.log(np.maximum(op * 255.0, 1e-300))
        r = np.sqrt(2.0 * np.maximum(ln_t, 0.0) * pp['lam'])
        r = np.where((ln_t > 0) & pp['valid'], r, 0.0) + 0.5
        live = pp['valid'] & (ln_t > 0)
        px, py = pp['px'].astype(np.float64), pp['py'].astype(np.float64)
        A = pp['cA'].astype(np.float64)
        Bc = pp['cB'].astype(np.float64)
        C = pp['cC'].astype(np.float64)
        lnop = np.log(np.maximum(op, 1e-300))
        rgbd4 = np.zeros((P, 4), np.float32)
        rgbd4[:, 0:3] = pp['rgb']
        rgbd4[:, 3] = np.float32(1.0) / pp['dep']
        for ty in range(nty):
            y0, y1 = ty * TS_Y, (ty + 1) * TS_Y - 1
            for tx in range(ntx):
                x0, x1 = tx * TS_X, (tx + 1) * TS_X - 1
                bbox = (live & (px + r >= x0) & (px - r <= x1)
                        & (py + r >= y0) & (py - r <= y1))
                # exact max of the (concave) power quadratic over the
                # tile rect; conservative vs the integer pixel grid
                dxl, dxh = x0 - px, x1 - px
                dyl, dyh = y0 - py, y1 - py
                inside = (dxl <= 0) & (dxh >= 0) & (dyl <= 0) & (dyh >= 0)
                best = np.where(inside, 0.0, -np.inf)
                for dx in (dxl, dxh):
                    ys = np.clip(-Bc * dx / C, dyl, dyh)
                    best = np.maximum(best, -0.5 * (A * dx * dx + C * ys * ys)
                                      - Bc * dx * ys)
                for dy in (dyl, dyh):
                    xs = np.clip(-Bc * dy / A, dxl, dxh)
                    best = np.maximum(best, -0.5 * (A * xs * xs + C * dy * dy)
                                      - Bc * xs * dy)
                lnth = np.log(1.0 / 255.0) - lnop
                sel = np.nonzero(bbox & (best >= lnth - 1e-3))[0]
                if sel.size == 0:
                    continue
                cx = x0 + (TS_X - 1) / 2.0
                cy = y0 + (TS_Y - 1) / 2.0
                pxl, pyl = px[sel] - cx, py[sel] - cy
                As, Bs, Cs = A[sel], Bc[sel], C[sel]
                c6 = np.stack([
                    -0.5 * As,
                    -0.5 * Cs,
                    -Bs,
                    As * pxl + Bs * pyl,
                    Cs * pyl + Bs * pxl,
                    -(0.5 * As * pxl * pxl + 0.5 * Cs * pyl * pyl
                      + Bs * pxl * pyl) + lnop[sel],
                ], axis=0)  # [6, n]
                n = sel.size
                for k in range((n + GRAN - 1) // GRAN):
                    s, e = k * GRAN, min((k + 1) * GRAN, n)
                    m = e - s
                    ct = np.zeros((6, GRAN), np.float32)
                    ct[5, :] = NEG_BIG
                    ct[:, :m] = c6[:, s:e].astype(np.float32)
                    r4 = np.zeros((GRAN, 4), np.float32)
                    r4[:m] = rgbd4[sel[s:e]]
                    wd = np.zeros((GRAN, 5), np.float32)
                    wd[0:GRAN - 1, 0:4] = r4[1:GRAN] - r4[0:GRAN - 1]
                    wd[GRAN - 1, 0:4] = -r4[GRAN - 1]
                    wd[GRAN - 1, 4] = 1.0   # T row: picks E[last]
                    groups.append(dict(key=(b, ty, tx, k), coef=ct, wd=wd,
                                       rfirst=r4[0].copy()))
    return groups


def _features():
    yy, xx = np.meshgrid(np.arange(TS_Y), np.arange(TS_X), indexing='ij')
    xl = (xx - (TS_X - 1) / 2.0).ravel()
    yl = (yy - (TS_Y - 1) / 2.0).ravel()
    return np.stack([xl * xl, yl * yl, xl * yl, xl, yl,
                     np.ones(NPIX)], axis=0).astype(np.float32)  # [6, NPIX]


_NC_CACHE = {}


def _build_nc(J):
    if J in _NC_CACHE:
        return _NC_CACHE[J]
    import concourse.bacc as bacc
    import concourse.mybir as mybir
    import concourse.tile as tile
    from concourse.tile import add_dep_helper
    from contextlib import ExitStack

    f32 = mybir.dt.float32
    Alu = mybir.AluOpType
    Act = mybir.ActivationFunctionType

    nc = bacc.Bacc("TRN2", target_bir_lowering=False, debug=False,
                   num_devices=NCORES)
    # batched layouts: one DMA each for coef / wout / out
    coef_d = nc.dram_tensor("coef", [J, 6, GB], f32, kind="ExternalInput")
    wout_d = nc.dram_tensor("wout", [J, GB, MOUT], f32, kind="ExternalInput")
    F_d = nc.dram_tensor("feat", [6, NPIX], f32, kind="ExternalInput")
    U_d = nc.dram_tensor("triu", [GB, GB], f32, kind="ExternalInput")
    out_d = nc.dram_tensor("out", [MOUT, J, NPIX], f32, kind="ExternalOutput")

    with tile.TileContext(nc) as tc, ExitStack() as ctx:
        pairs = [list(range(p, min(p + 2, J))) for p in range(0, J, 2)]
        NP = len(pairs)
        const = ctx.enter_context(tc.tile_pool(name="const", bufs=1))
        pool = ctx.enter_context(tc.tile_pool(name="work", bufs=3))
        lpool = ctx.enter_context(tc.tile_pool(name="lpool", bufs=NP))
        epool = ctx.enter_context(tc.tile_pool(name="epool", bufs=NP))
        acpool = ctx.enter_context(tc.tile_pool(name="acpool", bufs=NP))
        psum = ctx.enter_context(
            tc.tile_pool(name="psum", bufs=2, space="PSUM"))

        F_t = const.tile([6, NPIX], f32)
        nc.sync.dma_start(F_t[:], F_d[:])
        U_t = const.tile([GB, GB], f32)
        nc.sync.dma_start(U_t[:], U_d[:])
        coef_t = const.tile([6, J * GB], f32)
        nc.sync.dma_start(
            coef_t[:].rearrange("p (j g) -> p j g", j=J),
            coef_d[:].rearrange("j p g -> p j g"))
        wout_t = const.tile([GB, J * MOUT], f32)
        nc.sync.dma_start(
            wout_t[:].rearrange("p (j m) -> p j m", j=J),
            wout_d[:].rearrange("j p m -> p j m"))
        oall_t = const.tile([MOUT, J * NPIX], f32)

        alphas, acs, ls, es = ({} for _ in range(4))
        expA, lns, expC = [], [], []
        # phase A: feature matmuls (per job) + one exp per pair
        for p, pr in enumerate(pairs):
            wN = len(pr) * NPIX
            power_p = psum.tile([GB, 512], f32, tag="power",
                                name=f"power{p}")
            for o, j in enumerate(pr):
                nc.tensor.matmul(power_p[:, o * NPIX:(o + 1) * NPIX],
                                 coef_t[:, j * GB:(j + 1) * GB],
                                 F_t[:], start=True, stop=True)
            alphas[p] = pool.tile([GB, wN], f32, tag="alpha",
                                  name=f"alpha{p}")
            expA.append(nc.scalar.activation(alphas[p][:],
                                             power_p[:, 0:wN], Act.Exp))
        # phase B: DVE clamp+mask per pair, then Ln per pair
        for p, pr in enumerate(pairs):
            wN = len(pr) * NPIX
            amin_t = pool.tile([GB, wN], f32, tag="amin")
            nc.vector.tensor_scalar(amin_t[:], alphas[p][:], 0.99, None,
                                    Alu.min)
            mask_t = pool.tile([GB, wN], f32, tag="mask")
            nc.vector.tensor_scalar(mask_t[:], alphas[p][:], 1.0 / 255.0,
                                    None, Alu.is_ge)
            acs[p] = acpool.tile([GB, wN], f32, tag="ac", name=f"ac{p}")
            nc.vector.tensor_tensor(acs[p][:], amin_t[:], mask_t[:],
                                    Alu.mult)
        for p, pr in enumerate(pairs):
            wN = len(pr) * NPIX
            ls[p] = lpool.tile([GB, wN], f32, tag="l", name=f"l{p}")
            i = nc.scalar.activation(ls[p][:], acs[p][:], Act.Ln, bias=1.0,
                                     scale=-1.0)
            lns.append(i)
            add_dep_helper(i.ins, expA[-1].ins, sync=False,
                           reason="group ACT tables: Ln after phase-A Exps")
        # phase C: cumsum matmul + exp per pair
        for p, pr in enumerate(pairs):
            wN = len(pr) * NPIX
            cum_p = psum.tile([GB, 512], f32, tag="cum", name=f"cum{p}")
            nc.tensor.matmul(cum_p[:, 0:wN], U_t[:], ls[p][:],
                             start=True, stop=True)
            es[p] = epool.tile([GB, wN], f32, tag="e", name=f"e{p}")
            i = nc.scalar.activation(es[p][:], cum_p[:, 0:wN], Act.Exp)
            expC.append(i)
            add_dep_helper(i.ins, lns[-1].ins, sync=False,
                           reason="group ACT tables: Exp after all Ln")
        # phase D: per-job output matmul, copy into staging, one DMA out
        for p, pr in enumerate(pairs):
            for o, j in enumerate(pr):
                out_p = psum.tile([MOUT, NPIX], f32, tag="out")
                nc.tensor.matmul(out_p[:],
                                 wout_t[:, j * MOUT:(j + 1) * MOUT],
                                 es[p][:, o * NPIX:(o + 1) * NPIX],
                                 start=True, stop=True)
                nc.vector.tensor_copy(oall_t[:, j * NPIX:(j + 1) * NPIX],
                                      out_p[:])
        nc.sync.dma_start(
            out_d[:].rearrange("p j n -> p (j n)"), oall_t[:])

    nc.compile()
    _NC_CACHE[J] = nc
    return nc


def kernel(means3D, opacities, scales, rotations, sh, bg,
           viewmatrices, projmatrices, camposes, tanfovxs, tanfovys,
           _run_opts=None):
    from concourse.bass_utils import run_bass_kernel_spmd

    means3D = np.asarray(means3D, np.float32)
    opacities = np.asarray(opacities, np.float32)
    scales = np.asarray(scales, np.float32)
    rotations = np.asarray(rotations, np.float32)
    sh = np.asarray(sh, np.float32)
    bg = np.asarray(bg, np.float32)
    viewmatrices = np.asarray(viewmatrices, np.float32)
    projmatrices = np.asarray(projmatrices, np.float32)
    camposes = np.asarray(camposes, np.float32)
    tanfovxs = np.asarray(tanfovxs, np.float32)
    tanfovys = np.asarray(tanfovys, np.float32)

    pps = [_preprocess_one(viewmatrices[b], projmatrices[b], camposes[b],
                           float(tanfovxs[b]), float(tanfovys[b]),
                           means3D, opacities, scales, rotations, sh)
           for b in range(B)]
    groups = _build_groups(pps)

    njob = (len(groups) + NGRP - 1) // NGRP
    J = (njob + NCORES - 1) // NCORES
    # pad with dummy groups (alpha=0 everywhere, no output consumer)
    dummy = dict(key=None, coef=None, wd=None, rfirst=None)
    while len(groups) < J * NCORES * NGRP:
        groups.append(dummy)

    F = _features()
    U8 = np.triu(np.ones((GRAN, GRAN), np.float32))
    U = np.zeros((GB, GB), np.float32)
    for g in range(NGRP):
        U[g * GRAN:(g + 1) * GRAN, g * GRAN:(g + 1) * GRAN] = U8

    # pack per (core, job): coef [6,GB], wout [GB,MOUT]
    coef_all = np.zeros((NCORES, J, 6, GB), np.float32)
    coef_all[:, :, 5, :] = NEG_BIG
    wout_all = np.zeros((NCORES, J, GB, MOUT), np.float32)
    gmeta = {}  # (core, j, g) -> (key, rfirst)
    for i, gr in enumerate(groups):
        cj, g = divmod(i, NGRP)
        c, j = divmod(cj, J)
        if gr['key'] is None:
            continue
        coef_all[c, j, :, g * GRAN:(g + 1) * GRAN] = gr['coef']
        wout_all[c, j, g * GRAN:(g + 1) * GRAN, g * 5:(g + 1) * 5] = gr['wd']
        gmeta[(c, j, g)] = (gr['key'], gr['rfirst'])

    nc = _build_nc(J)
    in_maps = [{'coef': coef_all[c], 'wout': wout_all[c],
                'feat': F, 'triu': U} for c in range(NCORES)]
    run_opts = dict(_run_opts or {})
    result_sink = run_opts.pop('result_sink', None)
    res = run_bass_kernel_spmd(nc, in_maps, list(range(NCORES)), **run_opts)
    if result_sink is not None:
        result_sink['res'] = res
    outs = [res.results[c]['out'] for c in range(NCORES)]  # [MOUT,J,NPIX]

    # host combine: per (cam, tile) chain chunks in depth order
    nty, ntx = H // TS_Y, W // TS_X
    acc = {}
    for (c, j, g), (key, rfirst) in gmeta.items():
        b, ty, tx, chunk = key
        o = outs[c][g * 5:(g + 1) * 5, j]          # [5, NPIX]
        acc.setdefault((b, ty, tx), []).append((chunk, o, rfirst))
    colors = np.zeros((B, 3, H, W), np.float32)
    invd = np.zeros((B, 1, H, W), np.float32)
    for b in range(B):
        colors[b] = bg[:, None, None]
    for (b, ty, tx), chunks in acc.items():
        chunks.sort(key=lambda t: t[0])
        Csum = np.zeros((4, NPIX), np.float32)
        Trun = np.ones((NPIX,), np.float32)
        for _, o, rfirst in chunks:
            Csum = Csum + Trun[None, :] * (o[0:4] + rfirst[:, None])
            Trun = Trun * o[4]
        ybase, xbase = ty * TS_Y, tx * TS_X
        tilec = (Csum[0:3] + Trun[None, :] * bg[:, None]).reshape(
            3, TS_Y, TS_X)
        colors[b, :, ybase:ybase + TS_Y, xbase:xbase + TS_X] = tilec
        invd[b, 0, ybase:ybase + TS_Y, xbase:xbase + TS_X] = \
            Csum[3].reshape(TS_Y, TS_X)

    radii = np.stack([pp['radii_unsorted'] for pp in pps])
    return colors, invd, radii


# revision 17
# speedup vs baseline: 4.9755x; 1.0862x over previous
"""Batch 3D-Gaussian rasterizer on 8 Trainium2 NeuronCores.

Strategy: host does the cheap per-gaussian preprocess (projection, conic,
SH color, depth sort) and tile culling; the device does the heavy
pixel x gaussian rasterization. The image is cut into 16x32-pixel tiles;
each (tile, 128-gaussian depth block) becomes one independent job that
computes per-pixel (color partial, invdepth partial, block transmittance).
Jobs are distributed round-robin over the 8 cores; the host then combines
a tile's blocks in depth order with C = C0 + T0*C1 + ..., which is exact.

Per-job device pipeline (gaussian axis on SBUF partitions, pixels on free):
  power  = coef^T @ F            (PE, K=6 quadratic-form features)
  alpha  = exp(power)            (ACT; ln(opacity) folded into coef const)
  alpha_c= min(alpha,.99)*(alpha>=1/255)   (DVE: min, is_ge, mult)
  l      = ln(1 - alpha_c)       (ACT, scale=-1 bias=1 fused)
  cum    = U_incl^T @ l          (PE, triangular-ones inclusive cumsum)
  E      = exp(cum)              (ACT)  # E[g] = T_after(g); T_before(g)=E[g-1]
  out4  += rgbd^T @ T_before - rgbd^T @ E   (PE, signed matmuls; w = Tb-Ta)
  out    = [out4 rows; E[127] = block transmittance row]
"""
import sys
import numpy as np

for _p in ('/opt/trn_rl_repo',):
    if _p not in sys.path:
        sys.path.append(_p)

B, P, H, W = 2, 1024, 128, 128
SCALE_MOD = 1.0
NEAR = 0.2
TS_Y, TS_X = 16, 16          # image tile = 16x16 = 256 px
NPIX = TS_Y * TS_X
GB = 128                     # SBUF partition dim (16 groups of GRAN rows)
GRAN = 8                     # gaussians per group (tile depth sub-block)
NGRP = GB // GRAN            # groups per job
MOUT = NGRP * 5              # output rows: 4 accums + 1 T row per group
NCORES = 8
NEG_BIG = -1.0e30

_C0 = 0.28209479177387814
_C1 = 0.4886025119029199
_C2 = (1.0925484305920792, -1.0925484305920792, 0.31539156525252005,
       -1.0925484305920792, 0.5462742152960396)
_C3 = (-0.5900435899266435, 2.890611442640554, -0.4570457994644658,
       0.3731763325901154, -0.4570457994644658, 1.445305721320277,
       -0.5900435899266435)


def _eval_sh(sh, dirs):
    x, y, z = dirs[:, 0:1], dirs[:, 1:2], dirs[:, 2:3]
    res = _C0 * sh[:, 0]
    res = res - _C1 * y * sh[:, 1] + _C1 * z * sh[:, 2] - _C1 * x * sh[:, 3]
    xx, yy, zz = x * x, y * y, z * z
    xy, yz, xz = x * y, y * z, x * z
    res = (res + _C2[0] * xy * sh[:, 4] + _C2[1] * yz * sh[:, 5]
           + _C2[2] * (2.0 * zz - xx - yy) * sh[:, 6]
           + _C2[3] * xz * sh[:, 7] + _C2[4] * (xx - yy) * sh[:, 8])
    res = (res + _C3[0] * y * (3.0 * xx - yy) * sh[:, 9]
           + _C3[1] * xy * z * sh[:, 10]
           + _C3[2] * y * (4.0 * zz - xx - yy) * sh[:, 11]
           + _C3[3] * z * (2.0 * zz - 3.0 * xx - 3.0 * yy) * sh[:, 12]
           + _C3[4] * x * (4.0 * zz - xx - yy) * sh[:, 13]
           + _C3[5] * z * (xx - yy) * sh[:, 14]
           + _C3[6] * x * (xx - 3.0 * yy) * sh[:, 15])
    return np.maximum(res + 0.5, 0.0).astype(np.float32)


def _quat_to_rot(q):
    q = q / np.linalg.norm(q, axis=-1, keepdims=True)
    r, x, y, z = q[:, 0], q[:, 1], q[:, 2], q[:, 3]
    return np.stack([
        1 - 2 * (y * y + z * z), 2 * (x * y - r * z), 2 * (x * z + r * y),
        2 * (x * y + r * z), 1 - 2 * (x * x + z * z), 2 * (y * z - r * x),
        2 * (x * z - r * y), 2 * (y * z + r * x), 1 - 2 * (x * x + y * y),
    ], axis=-1).reshape(-1, 3, 3).astype(np.float32)


def _preprocess_one(viewmat, projmat, campos, tanx, tany,
                    means3D, opacities, scales, rotations, sh):
    Pn = means3D.shape[0]
    ones = np.ones((Pn, 1), means3D.dtype)
    p_hom = np.concatenate([means3D, ones], axis=1)
    t = (p_hom @ viewmat)[:, :3]
    p_proj = p_hom @ projmat
    ndc = p_proj[:, :3] / (p_proj[:, 3:4] + 1e-7)
    depth = t[:, 2]

    focal_x = W / (2.0 * tanx)
    focal_y = H / (2.0 * tany)
    tz = np.maximum(depth, 1e-6)
    txc = np.clip(t[:, 0] / tz, -1.3 * tanx, 1.3 * tanx) * tz
    tyc = np.clip(t[:, 1] / tz, -1.3 * tany, 1.3 * tany) * tz
    zero = np.zeros_like(tz)
    J = np.stack([
        np.stack([focal_x / tz, zero, -focal_x * txc / (tz * tz)], -1),
        np.stack([zero, focal_y / tz, -focal_y * tyc / (tz * tz)], -1),
    ], axis=1).astype(np.float32)
    Wr = viewmat[:3, :3].T
    Rq = _quat_to_rot(rotations)
    s2 = ((scales * SCALE_MOD) ** 2).astype(np.float32)
    Sigma = np.einsum('pij,pj,pkj->pik', Rq, s2, Rq).astype(np.float32)
    T = np.einsum('pij,jk->pik', J, Wr).astype(np.float32)
    cov2d = np.einsum('pij,pjk,plk->pil', T, Sigma, T).astype(np.float32)
    a = cov2d[:, 0, 0] + 0.3
    c = cov2d[:, 1, 1] + 0.3
    b = cov2d[:, 0, 1]
    det = a * c - b * b
    valid = (det > 0.0) & (depth > NEAR)
    det_s = np.where(valid, det, 1.0)
    conA, conB, conC = c / det_s, -b / det_s, a / det_s

    mid = 0.5 * (a + c)
    lam = mid + np.sqrt(np.maximum(0.1, mid * mid - det))
    radii = np.where(valid, np.ceil(3.0 * np.sqrt(lam)), 0.0).astype(np.int32)

    px = ((ndc[:, 0] + 1.0) * W - 1.0) * 0.5
    py = ((ndc[:, 1] + 1.0) * H - 1.0) * 0.5

    dirs = means3D - campos[None, :]
    dirs = dirs / np.linalg.norm(dirs, axis=-1, keepdims=True)
    rgb = _eval_sh(sh, dirs)

    order = np.argsort(depth, kind='stable')
    return dict(
        px=px[order], py=py[order],
        cA=conA[order], cB=conB[order], cC=conC[order],
        op=opacities[order, 0], rgb=rgb[order],
        dep=np.maximum(depth[order], 1e-6), valid=valid[order],
        lam=lam[order], radii_unsorted=radii,
    )


def _build_groups(pps):
    """Cut every (cam, tile) gaussian list into depth-ordered chunks of
    GRAN. Returns a list of group dicts: coef [6,GRAN], wd [GRAN,5]
    (diff-rgbd cols 0:4, T-indicator col 4), rfirst [4] (host-added
    constant term), and (b, ty, tx, chunk)."""
    nty, ntx = H // TS_Y, W // TS_X
    groups = []
    for b, pp in enumerate(pps):
        op = pp['op'].astype(np.float64)
        ln_t = np.log(np.maximum(op * 255.0, 1e-300))
        r = np.sqrt(2.0 * np.maximum(ln_t, 0.0) * pp['lam'])
        r = np.where((ln_t > 0) & pp['valid'], r, 0.0) + 0.5
        live = pp['valid'] & (ln_t > 0)
        px, py = pp['px'].astype(np.float64), pp['py'].astype(np.float64)
        A = pp['cA'].astype(np.float64)
        Bc = pp['cB'].astype(np.float64)
        C = pp['cC'].astype(np.float64)
        lnop = np.log(np.maximum(op, 1e-300))
        rgbd4 = np.zeros((P, 4), np.float32)
        rgbd4[:, 0:3] = pp['rgb']
        rgbd4[:, 3] = np.float32(1.0) / pp['dep']
        for ty in range(nty):
            y0, y1 = ty * TS_Y, (ty + 1) * TS_Y - 1
            for tx in range(ntx):
                x0, x1 = tx * TS_X, (tx + 1) * TS_X - 1
                bbox = (live & (px + r >= x0) & (px - r <= x1)
                        & (py + r >= y0) & (py - r <= y1))
                # exact max of the (concave) power quadratic over the
                # tile rect; conservative vs the integer pixel grid
                dxl, dxh = x0 - px, x1 - px
                dyl, dyh = y0 - py, y1 - py
                inside = (dxl <= 0) & (dxh >= 0) & (dyl <= 0) & (dyh >= 0)
                best = np.where(inside, 0.0, -np.inf)
                for dx in (dxl, dxh):
                    ys = np.clip(-Bc * dx / C, dyl, dyh)
                    best = np.maximum(best, -0.5 * (A * dx * dx + C * ys * ys)
                                      - Bc * dx * ys)
                for dy in (dyl, dyh):
                    xs = np.clip(-Bc * dy / A, dxl, dxh)
                    best = np.maximum(best, -0.5 * (A * xs * xs + C * dy * dy)
                                      - Bc * xs * dy)
                lnth = np.log(1.0 / 255.0) - lnop
                sel = np.nonzero(bbox & (best >= lnth - 1e-3))[0]
                if sel.size == 0:
                    continue
                cx = x0 + (TS_X - 1) / 2.0
                cy = y0 + (TS_Y - 1) / 2.0
                pxl, pyl = px[sel] - cx, py[sel] - cy
                As, Bs, Cs = A[sel], Bc[sel], C[sel]
                c6 = np.stack([
                    -0.5 * As,
                    -0.5 * Cs,
                    -Bs,
                    As * pxl + Bs * pyl,
                    Cs * pyl + Bs * pxl,
                    -(0.5 * As * pxl * pxl + 0.5 * Cs * pyl * pyl
                      + Bs * pxl * pyl) + lnop[sel],
                ], axis=0)  # [6, n]
                n = sel.size
                for k in range((n + GRAN - 1) // GRAN):
                    s, e = k * GRAN, min((k + 1) * GRAN, n)
                    m = e - s
                    ct = np.zeros((6, GRAN), np.float32)
                    ct[5, :] = NEG_BIG
                    ct[:, :m] = c6[:, s:e].astype(np.float32)
                    r4 = np.zeros((GRAN, 4), np.float32)
                    r4[:m] = rgbd4[sel[s:e]]
                    wd = np.zeros((GRAN, 5), np.float32)
                    wd[0:GRAN - 1, 0:4] = r4[1:GRAN] - r4[0:GRAN - 1]
                    wd[GRAN - 1, 0:4] = -r4[GRAN - 1]
                    wd[GRAN - 1, 4] = 1.0   # T row: picks E[last]
                    groups.append(dict(key=(b, ty, tx, k), coef=ct, wd=wd,
                                       rfirst=r4[0].copy()))
    return groups


def _features():
    yy, xx = np.meshgrid(np.arange(TS_Y), np.arange(TS_X), indexing='ij')
    xl = (xx - (TS_X - 1) / 2.0).ravel()
    yl = (yy - (TS_Y - 1) / 2.0).ravel()
    return np.stack([xl * xl, yl * yl, xl * yl, xl, yl,
                     np.ones(NPIX)], axis=0).astype(np.float32)  # [6, NPIX]


_NC_CACHE = {}


def _build_nc(J):
    if J in _NC_CACHE:
        return _NC_CACHE[J]
    import concourse.bacc as bacc
    import concourse.mybir as mybir
    import concourse.tile as tile
    from concourse.tile import add_dep_helper
    from contextlib import ExitStack

    f32 = mybir.dt.float32
    f32r = mybir.dt.float32r
    Alu = mybir.AluOpType
    Act = mybir.ActivationFunctionType

    nc = bacc.Bacc("TRN2", target_bir_lowering=False, debug=False,
                   num_devices=NCORES)
    # batched layouts: one DMA each for coef / wout / out
    coef_d = nc.dram_tensor("coef", [J, 6, GB], f32, kind="ExternalInput")
    wout_d = nc.dram_tensor("wout", [J, GB, MOUT], f32, kind="ExternalInput")
    F_d = nc.dram_tensor("feat", [6, NPIX], f32, kind="ExternalInput")
    U_d = nc.dram_tensor("triu", [GB, GB], f32, kind="ExternalInput")
    out_d = nc.dram_tensor("out", [MOUT, J, NPIX], f32, kind="ExternalOutput")

    with tile.TileContext(nc) as tc, ExitStack() as ctx:
        pairs = [list(range(p, min(p + 2, J))) for p in range(0, J, 2)]
        NP = len(pairs)
        const = ctx.enter_context(tc.tile_pool(name="const", bufs=1))
        pool = ctx.enter_context(tc.tile_pool(name="work", bufs=3))
        lpool = ctx.enter_context(tc.tile_pool(name="lpool", bufs=NP))
        epool = ctx.enter_context(tc.tile_pool(name="epool", bufs=NP))
        acpool = ctx.enter_context(tc.tile_pool(name="acpool", bufs=NP))
        psum = ctx.enter_context(
            tc.tile_pool(name="psum", bufs=2, space="PSUM"))

        F_t = const.tile([6, NPIX], f32)
        nc.sync.dma_start(F_t[:], F_d[:])
        coef_t = const.tile([6, J * GB], f32)
        nc.sync.dma_start(
            coef_t[:].rearrange("p (j g) -> p j g", j=J),
            coef_d[:].rearrange("j p g -> p j g"))
        U_t = const.tile([GB, GB], f32r)
        nc.gpsimd.dma_start(U_t[:], U_d[:])
        wout_t = const.tile([GB, J * MOUT], f32r)
        nc.gpsimd.dma_start(
            wout_t[:].rearrange("p (j m) -> p j m", j=J),
            wout_d[:].rearrange("j p m -> p j m"))
        oall_t = const.tile([MOUT, J * NPIX], f32)

        alphas, acs, ls, es = ({} for _ in range(4))
        expA, lns, expC = [], [], []
        # phase A: feature matmuls (per job) + one exp per pair
        for p, pr in enumerate(pairs):
            wN = len(pr) * NPIX
            power_p = psum.tile([GB, 512], f32, tag="power",
                                name=f"power{p}")
            for o, j in enumerate(pr):
                nc.tensor.matmul(power_p[:, o * NPIX:(o + 1) * NPIX],
                                 coef_t[:, j * GB:(j + 1) * GB],
                                 F_t[:], start=True, stop=True)
            alphas[p] = pool.tile([GB, wN], f32, tag="alpha",
                                  name=f"alpha{p}")
            expA.append(nc.scalar.activation(alphas[p][:],
                                             power_p[:, 0:wN], Act.Exp))
        # phase B: DVE clamp+mask per pair, then Ln per pair
        for p, pr in enumerate(pairs):
            wN = len(pr) * NPIX
            amin_t = pool.tile([GB, wN], f32, tag="amin")
            nc.vector.tensor_scalar(amin_t[:], alphas[p][:], 0.99, None,
                                    Alu.min)
            mask_t = pool.tile([GB, wN], f32, tag="mask")
            nc.vector.tensor_scalar(mask_t[:], alphas[p][:], 1.0 / 255.0,
                                    None, Alu.is_ge)
            acs[p] = acpool.tile([GB, wN], f32, tag="ac", name=f"ac{p}")
            nc.vector.tensor_tensor(acs[p][:], amin_t[:], mask_t[:],
                                    Alu.mult)
        for p, pr in enumerate(pairs):
            wN = len(pr) * NPIX
            ls[p] = lpool.tile([GB, wN], f32r, tag="l", name=f"l{p}")
            i = nc.scalar.activation(ls[p][:], acs[p][:], Act.Ln, bias=1.0,
                                     scale=-1.0)
            lns.append(i)
            add_dep_helper(i.ins, expA[-1].ins, sync=False,
                           reason="group ACT tables: Ln after phase-A Exps")
        # phase C: cumsum matmul + exp per pair
        for p, pr in enumerate(pairs):
            wN = len(pr) * NPIX
            cum_p = psum.tile([GB, 512], f32, tag="cum", name=f"cum{p}")
            nc.tensor.matmul(cum_p[:, 0:wN], U_t[:], ls[p][:],
                             start=True, stop=True)
            es[p] = epool.tile([GB, wN], f32r, tag="e", name=f"e{p}")
            i = nc.scalar.activation(es[p][:], cum_p[:, 0:wN], Act.Exp)
            expC.append(i)
            add_dep_helper(i.ins, lns[-1].ins, sync=False,
                           reason="group ACT tables: Exp after all Ln")
        # phase D: per-job output matmul, copy into staging, one DMA out
        for p, pr in enumerate(pairs):
            for o, j in enumerate(pr):
                out_p = psum.tile([MOUT, NPIX], f32, tag="out")
                nc.tensor.matmul(out_p[:],
                                 wout_t[:, j * MOUT:(j + 1) * MOUT],
                                 es[p][:, o * NPIX:(o + 1) * NPIX],
                                 start=True, stop=True)
                nc.vector.tensor_copy(oall_t[:, j * NPIX:(j + 1) * NPIX],
                                      out_p[:])
        nc.sync.dma_start(
            out_d[:].rearrange("p j n -> p (j n)"), oall_t[:])

    nc.compile()
    _NC_CACHE[J] = nc
    return nc


def kernel(means3D, opacities, scales, rotations, sh, bg,
           viewmatrices, projmatrices, camposes, tanfovxs, tanfovys,
           _run_opts=None):
    from concourse.bass_utils import run_bass_kernel_spmd

    means3D = np.asarray(means3D, np.float32)
    opacities = np.asarray(opacities, np.float32)
    scales = np.asarray(scales, np.float32)
    rotations = np.asarray(rotations, np.float32)
    sh = np.asarray(sh, np.float32)
    bg = np.asarray(bg, np.float32)
    viewmatrices = np.asarray(viewmatrices, np.float32)
    projmatrices = np.asarray(projmatrices, np.float32)
    camposes = np.asarray(camposes, np.float32)
    tanfovxs = np.asarray(tanfovxs, np.float32)
    tanfovys = np.asarray(tanfovys, np.float32)

    pps = [_preprocess_one(viewmatrices[b], projmatrices[b], camposes[b],
                           float(tanfovxs[b]), float(tanfovys[b]),
                           means3D, opacities, scales, rotations, sh)
           for b in range(B)]
    groups = _build_groups(pps)

    njob = (len(groups) + NGRP - 1) // NGRP
    J = (njob + NCORES - 1) // NCORES
    # pad with dummy groups (alpha=0 everywhere, no output consumer)
    dummy = dict(key=None, coef=None, wd=None, rfirst=None)
    while len(groups) < J * NCORES * NGRP:
        groups.append(dummy)

    F = _features()
    U8 = np.triu(np.ones((GRAN, GRAN), np.float32))
    U = np.zeros((GB, GB), np.float32)
    for g in range(NGRP):
        U[g * GRAN:(g + 1) * GRAN, g * GRAN:(g + 1) * GRAN] = U8

    # pack per (core, job): coef [6,GB], wout [GB,MOUT]
    coef_all = np.zeros((NCORES, J, 6, GB), np.float32)
    coef_all[:, :, 5, :] = NEG_BIG
    wout_all = np.zeros((NCORES, J, GB, MOUT), np.float32)
    gmeta = {}  # (core, j, g) -> (key, rfirst)
    for i, gr in enumerate(groups):
        cj, g = divmod(i, NGRP)
        c, j = divmod(cj, J)
        if gr['key'] is None:
            continue
        coef_all[c, j, :, g * GRAN:(g + 1) * GRAN] = gr['coef']
        wout_all[c, j, g * GRAN:(g + 1) * GRAN, g * 5:(g + 1) * 5] = gr['wd']
        gmeta[(c, j, g)] = (gr['key'], gr['rfirst'])

    nc = _build_nc(J)
    in_maps = [{'coef': coef_all[c], 'wout': wout_all[c],
                'feat': F, 'triu': U} for c in range(NCORES)]
    run_opts = dict(_run_opts or {})
    result_sink = run_opts.pop('result_sink', None)
    res = run_bass_kernel_spmd(nc, in_maps, list(range(NCORES)), **run_opts)
    if result_sink is not None:
        result_sink['res'] = res
    outs = [res.results[c]['out'] for c in range(NCORES)]  # [MOUT,J,NPIX]

    # host combine: per (cam, tile) chain chunks in depth order
    nty, ntx = H // TS_Y, W // TS_X
    acc = {}
    for (c, j, g), (key, rfirst) in gmeta.items():
        b, ty, tx, chunk = key
        o = outs[c][g * 5:(g + 1) * 5, j]          # [5, NPIX]
        acc.setdefault((b, ty, tx), []).append((chunk, o, rfirst))
    colors = np.zeros((B, 3, H, W), np.float32)
    invd = np.zeros((B, 1, H, W), np.float32)
    for b in range(B):
        colors[b] = bg[:, None, None]
    for (b, ty, tx), chunks in acc.items():
        chunks.sort(key=lambda t: t[0])
        Csum = np.zeros((4, NPIX), np.float32)
        Trun = np.ones((NPIX,), np.float32)
        for _, o, rfirst in chunks:
            Csum = Csum + Trun[None, :] * (o[0:4] + rfirst[:, None])
            Trun = Trun * o[4]
        ybase, xbase = ty * TS_Y, tx * TS_X
        tilec = (Csum[0:3] + Trun[None, :] * bg[:, None]).reshape(
            3, TS_Y, TS_X)
        colors[b, :, ybase:ybase + TS_Y, xbase:xbase + TS_X] = tilec
        invd[b, 0, ybase:ybase + TS_Y, xbase:xbase + TS_X] = \
            Csum[3].reshape(TS_Y, TS_X)

    radii = np.stack([pp['radii_unsorted'] for pp in pps])
    return colors, invd, radii
